# revision 1
# baseline (speedup 1.0000x reference)
"""Trainium2 Bass kernel for the sparse-attention ('interact' mask) transformer block.

Reference computation (B=4, N=1569, C=768, H=12, d=64, Dff=3072, F=9):
    h   = LN(x);  qkv = h @ qkv_w.T;  sparse attention (spatial rows attend
    only to the 9 temporal tokens, temporal rows attend to the 1560 spatial
    tokens, CLS also to itself);  out = attn @ proj_w.T + proj_b;
    return out + MLP(LN(out))

Sharding: 8 cores = 4 batches x 2 halves. Each core owns one batch's half of
the 1560 spatial tokens (780) plus a replicated copy of the 9 temporal
tokens; local token layout is [780 spatial | 9 temporal].  The only
communication is a pairwise AllReduce(add) of flash-style partial softmax
stats (l2 [108,1], O2 [9,768]) for the 9 temporal query rows.

On-chip layout is feature-major [C, tokens]; the host pre-transposes x and
all weights (pure data movement, part of sharding).  Matmuls run as fp32r
(full PE rate); LN statistics come from ones-matmuls; softmax skips the max
subtraction (scores are O(1) here, exp is safe in fp32).
"""

import numpy as np
import sys
from contextlib import ExitStack

sys.path.insert(0, '/opt/trn_rl_repo')

import concourse.bass as bass
import concourse.bacc as bacc
import concourse.tile as tile
from concourse import mybir
from concourse.bass_utils import run_bass_kernel_spmd

# ---------------- problem constants (hardcoded per contract) ----------------
B, N, C = 4, 1569, 768
H, D = 12, 64
F = 9                    # temporal tokens (CLS + 8 frames)
DFF = 4 * C              # 3072
NSP = N - F              # 1560 spatial tokens
SPH = NSP // 2           # 780 spatial tokens per core
T = SPH + F + 1          # 790 local cols: [780 spatial | 9 temporal | 1 zero pad]
                         # (pad keeps every fp32r matmul moving-dim even)
NCH = C // 128           # 6 feature chunks
NCH_FF = DFF // 128      # 24 hidden chunks
NTB = (T + 127) // 128   # 7 token blocks (last = 21 rows)
SCALE = D ** -0.5

FP32 = mybir.dt.float32
FP32R = mybir.dt.float32r
BF16 = mybir.dt.bfloat16

# free-dim tiles for matmul moving operand (<=512 fp32 / PSUM bank)
T_TILES = [(0, 512), (512, SPH), (SPH, T)]          # [0:512) [512:780) [780:790)
T_TILES_SP = [(0, 512), (512, SPH)]                 # spatial-only part


def _r(ap):
    """View an fp32 AP as fp32r for full-rate PE matmuls."""
    return ap.bitcast(FP32R)


def build_kernel():
    nc = bacc.Bacc("TRN2", target_bir_lowering=False, debug=False,
                   num_devices=8)

    # ---------------- DRAM I/O ----------------
    xT = nc.dram_tensor("xT", [C, T], FP32R, kind="ExternalInput")
    qkvWt = nc.dram_tensor("qkvWt", [C, 3 * C], BF16, kind="ExternalInput")
    projWt = nc.dram_tensor("projWt", [C, C], BF16, kind="ExternalInput")
    fc1Wt = nc.dram_tensor("fc1Wt", [C, DFF], BF16, kind="ExternalInput")
    fc2Wt = nc.dram_tensor("fc2Wt", [DFF, C], BF16, kind="ExternalInput")
    # [C,2]: col0 = ln2_g, col1 = ln2_b ; biases as [dim,1]
    gb = nc.dram_tensor("gb", [C, 2], FP32, kind="ExternalInput")
    projB = nc.dram_tensor("projB", [C, 1], FP32, kind="ExternalInput")
    fc1B = nc.dram_tensor("fc1B", [DFF, 1], FP32, kind="ExternalInput")
    fc2B = nc.dram_tensor("fc2B", [C, 1], FP32, kind="ExternalInput")
    ones = nc.dram_tensor("ones", [128, 1], FP32R, kind="ExternalInput")
    onesrow = nc.dram_tensor("onesrow", [1, 128], FP32R, kind="ExternalInput")
    headsel = nc.dram_tensor("headsel", [H, C], FP32R, kind="ExternalInput")
    bd9 = nc.dram_tensor("bd9", [H * F, H], FP32R, kind="ExternalInput")
    ident = nc.dram_tensor("ident", [128, 128], FP32, kind="ExternalInput")
    # e00mask [1,108]: is_even at positions h*9, else 0 (0 everywhere on odd cores)
    e00mask = nc.dram_tensor("e00mask", [1, H * F], FP32, kind="ExternalInput")
    zeros = nc.dram_tensor("zeros", [128, C], FP32R, kind="ExternalInput")
    outT = nc.dram_tensor("outT", [C, T], FP32, kind="ExternalOutput")

    with tile.TileContext(nc) as tc, ExitStack() as ctx:
        act = ctx.enter_context(tc.tile_pool(name="act", bufs=1))
        big = ctx.enter_context(tc.tile_pool(name="big", bufs=1))
        wpool = ctx.enter_context(tc.tile_pool(name="w", bufs=6))
        wpool2 = ctx.enter_context(tc.tile_pool(name="w2", bufs=25))
        small = ctx.enter_context(tc.tile_pool(name="small", bufs=1))
        stage = ctx.enter_context(tc.tile_pool(name="stage", bufs=2))
        psmm = ctx.enter_context(tc.tile_pool(name="psmm", bufs=3, space="PSUM"))
        psst = ctx.enter_context(tc.tile_pool(name="psst", bufs=3, space="PSUM"))
        pso2 = ctx.enter_context(tc.tile_pool(name="pso2", bufs=2, space="PSUM"))
        dram = ctx.enter_context(tc.tile_pool(name="dram", bufs=1, space="DRAM"))

        # ---------------- constants / biases ----------------
        ones_t = small.tile([128, 1], FP32R, tag="ones", name="ones")
        nc.sync.dma_start(ones_t[:], ones[:])
        onesrow_t = small.tile([1, 128], FP32R, tag="onesrow", name="onesrow")
        nc.sync.dma_start(onesrow_t[:], onesrow[:])
        headsel_t = small.tile([H, C], FP32R, tag="headsel", name="headsel")
        nc.sync.dma_start(headsel_t[:], headsel[:])
        bd9_t = small.tile([H * F, H], FP32R, tag="bd9", name="bd9")
        nc.sync.dma_start(bd9_t[:], bd9[:])
        id_t = small.tile([128, 128], FP32, tag="ident", name="ident")
        nc.sync.dma_start(id_t[:], ident[:])
        e00_t = small.tile([1, H * F], FP32, tag="e00", name="e00")
        nc.sync.dma_start(e00_t[:], e00mask[:])
        gb_t = [small.tile([128, 2], FP32, tag=f"gb{ci}", name=f"gb{ci}") for ci in range(NCH)]
        for ci in range(NCH):
            nc.sync.dma_start(gb_t[ci][:], gb[ci * 128:(ci + 1) * 128, :])
        pb_t = [small.tile([128, 1], FP32, tag=f"pb{ci}", name=f"pb{ci}") for ci in range(NCH)]
        for ci in range(NCH):
            nc.sync.dma_start(pb_t[ci][:], projB[ci * 128:(ci + 1) * 128, :])
        f1b_t = [small.tile([128, 1], FP32, tag=f"f1b{ci}", name=f"f1b{ci}") for ci in range(NCH_FF)]
        for ci in range(NCH_FF):
            nc.sync.dma_start(f1b_t[ci][:], fc1B[ci * 128:(ci + 1) * 128, :])
        f2b_t = [small.tile([128, 1], FP32, tag=f"f2b{ci}", name=f"f2b{ci}") for ci in range(NCH)]
        for ci in range(NCH):
            nc.sync.dma_start(f2b_t[ci][:], fc2B[ci * 128:(ci + 1) * 128, :])

        # ---------------- load x ----------------
        x_t = [act.tile([128, T], FP32R, tag=f"x{ci}", name=f"x{ci}") for ci in range(NCH)]
        for ci in range(NCH):
            nc.sync.dma_start(x_t[ci][:], xT[ci * 128:(ci + 1) * 128, :])

        # =========================================================
        # helper: layernorm stats + apply  (feature-major)
        #   in:  src chunks [128, T] x6     out: dst chunks [128, T] x6
        # =========================================================
        def layer_norm_fm(src, dst, scratch_tag, bc_a, bc_b):
            """LN over features (partition dim).  src/dst: 6 chunks [128,T].
            bc_a/bc_b: [128,T] scratch tiles for broadcast alpha/beta."""
            # x^2 into scratch
            sq = [act.tile([128, T], FP32R, tag=f"{scratch_tag}{ci}", name=f"{scratch_tag}{ci}")
                  for ci in range(NCH)]
            for ci in range(NCH):
                nc.scalar.activation(sq[ci][:], src[ci][:],
                                     mybir.ActivationFunctionType.Square)
            # LN scalar math stays in PSUM at partition 0 (ACT/DVE cannot
            # shift partitions; separate SBUF rows would break that rule).
            al_t = small.tile([1, T], FP32R, tag=f"{scratch_tag}_al", name=f"{scratch_tag}_al")
            be_t = small.tile([1, T], FP32R, tag=f"{scratch_tag}_be", name=f"{scratch_tag}_be")
            for (t0, t1) in T_TILES:
                w = t1 - t0
                ps = psst.tile([1, 512], FP32, tag="stat", name="stat")
                for ci in range(NCH):
                    nc.tensor.matmul(ps[:, :w], ones_t[:],
                                     src[ci][:, t0:t1],
                                     start=(ci == 0), stop=(ci == NCH - 1))
                ps2 = psst.tile([1, 512], FP32, tag="stat", name="stat2")
                for ci in range(NCH):
                    nc.tensor.matmul(ps2[:, :w], ones_t[:],
                                     sq[ci][:, t0:t1],
                                     start=(ci == 0), stop=(ci == NCH - 1))
                # scalar chain: one PSUM operand per op; intermediates in SBUF
                rowA = small.tile([1, 512], FP32, tag="lnA", name="lnA")
                rowB = small.tile([1, 512], FP32, tag="lnB", name="lnB")
                nc.vector.tensor_scalar_mul(ps[:, :w], ps[:, :w], 1.0 / C)
                nc.scalar.copy(rowA[:, :w], ps[:, :w])                  # mean
                nc.vector.tensor_scalar_mul(ps2[:, :w], ps2[:, :w], 1.0 / C)
                nc.vector.tensor_mul(rowB[:, :w], rowA[:, :w], rowA[:, :w])
                nc.vector.tensor_sub(ps2[:, :w], ps2[:, :w], rowB[:, :w])
                nc.vector.tensor_scalar_add(ps2[:, :w], ps2[:, :w], 1e-5)
                nc.scalar.activation(rowB[:, :w], ps2[:, :w],
                                     mybir.ActivationFunctionType.Sqrt)
                with nc.allow_low_precision(reason="fp32r LN alpha rounding intended"):
                    nc.vector.reciprocal(al_t[:, t0:t1], rowB[:, :w])
                nc.vector.tensor_mul(rowB[:, :w], rowA[:, :w], al_t[:, t0:t1])
                with nc.allow_low_precision(reason="fp32r LN beta rounding intended"):
                    nc.vector.tensor_scalar_mul(be_t[:, t0:t1], rowB[:, :w], -1.0)
            # broadcast alpha/beta across partitions via K=1 ones-matmul
            for (srow, bct) in ((al_t, bc_a), (be_t, bc_b)):
                for (t0, t1) in T_TILES:
                    psb = psmm.tile([128, 512], FP32, tag="mm", name="mm")
                    nc.tensor.matmul(psb[:, :t1 - t0], onesrow_t[:],
                                     srow[:, t0:t1],
                                     start=True, stop=True)
                    nc.scalar.copy(bct[:, t0:t1], psb[:, :t1 - t0])
            # apply: dst = (src*alpha + beta) * g + b
            for ci in range(NCH):
                nc.vector.tensor_mul(dst[ci][:], src[ci][:], bc_a[:])
                nc.vector.tensor_tensor(dst[ci][:], dst[ci][:], bc_b[:],
                                        op=mybir.AluOpType.add)
                nc.vector.tensor_scalar(dst[ci][:], dst[ci][:],
                                        gb_t[ci][:, 0:1], gb_t[ci][:, 1:2],
                                        op0=mybir.AluOpType.mult,
                                        op1=mybir.AluOpType.add)

        # =========================================================
        # STAGE A: LN1 + qkv
        # =========================================================
        h_t = [act.tile([128, T], BF16, tag=f"h{ci}", name=f"h{ci}") for ci in range(NCH)]
        bc_a = small.tile([128, T], FP32, tag="bca", name="bca")
        bc_b = small.tile([128, T], FP32, tag="bcb", name="bcb")
        layer_norm_fm(x_t, h_t, "k", bc_a, bc_b)  # scratch shares k-tag slots

        # q, k feature-major [C, T]; v token-major [T, C]
        q_t = [act.tile([128, T], FP32R, tag=f"q{ci}", name=f"q{ci}") for ci in range(NCH)]
        k_t = [act.tile([128, T], FP32R, tag=f"k{ci}", name=f"k{ci}") for ci in range(NCH)]
        v_t = [big.tile([128, C], FP32R, tag=f"v{tb}", name=f"v{tb}") for tb in range(NTB)]

        # q,k: for each 512-wide cout group load W [128,512] x6, then mm per 128-col block
        for qk in range(2):          # 0 = q, 1 = k
            dstl = q_t if qk == 0 else k_t
            for cg in range(0, C, 512):
                gw = min(512, C - cg)
                wts = [wpool.tile([128, 512], BF16, tag="w", name="w") for _ in range(NCH)]
                for ci in range(NCH):
                    nc.sync.dma_start(
                        wts[ci][:, :gw], qkvWt[ci * 128:(ci + 1) * 128,
                                               qk * C + cg: qk * C + cg + gw])
                for co in range(gw // 128):  # 128-col blocks within the group
                    cout = cg + co * 128
                    for (t0, t1) in T_TILES:
                        ps = psmm.tile([128, 512], FP32, tag="mm", name="mm")
                        for ci in range(NCH):
                            nc.tensor.matmul(
                                ps[:, :t1 - t0],
                                wts[ci][:, co * 128:(co + 1) * 128],
                                h_t[ci][:, t0:t1],
                                start=(ci == 0), stop=(ci == NCH - 1))
                        nc.scalar.copy(dstl[cout // 128][:, t0:t1], ps[:, :t1 - t0])

        # v token-major: for tok block: lhsT = h chunk [128cin, tb 128], rhs = W [128,512]
        for cg in range(0, C, 512):
            gw = min(512, C - cg)
            wts = [wpool.tile([128, 512], BF16, tag="w", name="w") for _ in range(NCH)]
            for ci in range(NCH):
                nc.sync.dma_start(
                    wts[ci][:, :gw], qkvWt[ci * 128:(ci + 1) * 128,
                                           2 * C + cg: 2 * C + cg + gw])
            for tb in range(NTB):
                p0, p1 = tb * 128, min((tb + 1) * 128, T)
                ps = psmm.tile([128, 512], FP32, tag="mm", name="mm")
                for ci in range(NCH):
                    nc.tensor.matmul(ps[:p1 - p0, :gw],
                                     h_t[ci][:, p0:p1],
                                     wts[ci][:, :gw],
                                     start=(ci == 0), stop=(ci == NCH - 1))
                nc.scalar.copy(v_t[tb][:p1 - p0, cg:cg + gw], ps[:p1 - p0, :gw])

        # =========================================================
        # STAGE B: sparse attention
        # =========================================================
        # temporal-token copies with base-partition 0 (tokens 780..789 live in
        # token block 6 at partitions 12..21)
        vtmp = small.tile([F, C], FP32R, tag="vtmp", name="vtmp")
        nc.sync.dma_start(vtmp[:], v_t[6][12:12 + F, :])

        # block-diag K_bd, Q_bd [128, 108] x6 : head h occupies rows (h%2)*64..,
        # cols h*9..h*9+9, from the temporal slice of k / q
        kbd = [small.tile([128, H * F], FP32R, tag=f"kbd{ci}", name=f"kbd{ci}") for ci in range(NCH)]
        qbd = [small.tile([128, H * F], FP32R, tag=f"qbd{ci}", name=f"qbd{ci}") for ci in range(NCH)]
        for ci in range(NCH):
            nc.sync.dma_start(kbd[ci][:], zeros[:, :H * F])
            nc.sync.dma_start(qbd[ci][:], zeros[:, :H * F])
        for h in range(H):
            ci, po = h // 2, (h % 2) * 64
            nc.vector.tensor_copy(kbd[ci][po:po + 64, h * F:(h + 1) * F],
                                  k_t[ci][po:po + 64, SPH:SPH + F])
            nc.vector.tensor_copy(qbd[ci][po:po + 64, h * F:(h + 1) * F],
                                  q_t[ci][po:po + 64, SPH:SPH + F])

        # ---- S1/P1: all local queries vs 9 temporal keys -> [108, T]
        p1 = small.tile([H * F, T], FP32R, tag="p1", name="p1")
        for (t0, t1) in T_TILES:
            ps = psmm.tile([128, 512], FP32, tag="mm", name="mm")
            for ci in range(NCH):
                nc.tensor.matmul(ps[:H * F, :t1 - t0], kbd[ci][:],
                                 q_t[ci][:, t0:t1],
                                 start=(ci == 0), stop=(ci == NCH - 1))
            nc.scalar.activation(p1[:, t0:t1], ps[:H * F, :t1 - t0],
                                 mybir.ActivationFunctionType.Exp, scale=SCALE)

        # lsp[h,t] = sum_j P1[h*9+j, t]  (fp32 matmul with block-diag ones)
        lsp = small.tile([H, T], FP32R, tag="lsp", name="lsp")
        for (t0, t1) in T_TILES:
            ps = psst.tile([12, 512], FP32, tag="stat", name="lspps")
            nc.tensor.matmul(ps[:, :t1 - t0], bd9_t[:], p1[:, t0:t1],
                             start=True, stop=True)
            nc.scalar.copy(lsp[:, t0:t1], ps[:, :t1 - t0])
        rlsp = lsp
        with nc.allow_low_precision(reason="fp32r rounding of softmax recip is intentional"):
            nc.vector.reciprocal(rlsp[:], lsp[:])

        # ---- O1: spatial attention out via block-diag v_tmp, normalized on copy
        # vtmp_bd [108, 768]: rows (h,j), head h's v at cols h*64..h*64+64
        vtmp_bd = small.tile([H * F, C], FP32R, tag="vtmpbd", name="vtmpbd")
        nc.sync.dma_start(vtmp_bd[:], zeros[:H * F, :])
        for h in range(H):
            nc.sync.dma_start(vtmp_bd[h * F:(h + 1) * F, h * 64:(h + 1) * 64],
                              vtmp[:, h * 64:(h + 1) * 64])
        attnout = [act.tile([128, T], BF16, tag=f"x{ci}", name=f"attn{ci}") for ci in range(NCH)]  # reuse x slots
        for ci in range(NCH):
            for (t0, t1) in T_TILES_SP:
                ps = psmm.tile([128, 512], FP32, tag="mm", name="mm")
                nc.tensor.matmul(ps[:, :t1 - t0],
                                 vtmp_bd[:, ci * 128:(ci + 1) * 128],
                                 p1[:, t0:t1],
                                 start=True, stop=True)
                # rl broadcast [128, t]: rows = rlsp[head(partition)]
                psr = psmm.tile([128, 512], FP32, tag="mm", name="mmrl")
                nc.tensor.matmul(psr[:, :t1 - t0],
                                 headsel_t[:, ci * 128:(ci + 1) * 128],
                                 rlsp[:, t0:t1],
                                 start=True, stop=True)
                bct = stage.tile([128, 512], FP32, tag="bc", name="bct")
                nc.scalar.copy(bct[:, :t1 - t0], psr[:, :t1 - t0])
                nc.vector.tensor_mul(attnout[ci][:, t0:t1], ps[:, :t1 - t0],
                                     bct[:, :t1 - t0])

        # ---- S2T/P2T: temporal queries vs all local keys, token-major [T, 108]
        p2 = [small.tile([128, H * F], FP32R, tag=f"p2{tb}", name=f"p2{tb}") for tb in range(NTB)]
        for tb in range(NTB):
            p0, p1_ = tb * 128, min((tb + 1) * 128, T)
            ps = psmm.tile([128, 512], FP32, tag="mm", name="mm")
            for ci in range(NCH):
                nc.tensor.matmul(ps[:p1_ - p0, :H * F],
                                 k_t[ci][:, p0:p1_], qbd[ci][:],
                                 start=(ci == 0), stop=(ci == NCH - 1))
            nc.scalar.activation(p2[tb][:p1_ - p0, :], ps[:p1_ - p0, :H * F],
                                 mybir.ActivationFunctionType.Exp, scale=SCALE)
        # temporal keys masked out; CLS self-term kept only on even cores (via
        # host-zeroed e00mask), only for query j=0: rows 780..789 sit in block 6
        # at partitions 12..21
        # replacement block for the 9 temporal-key rows: all zero except the
        # CLS self-term at (row 0, cols h*9) kept on even cores via e00mask
        e00tmp = small.tile([1, H * F], FP32R, tag="e00tmp", name="e00tmp")
        znine = small.tile([F + 1, H * F], FP32R, tag="znine", name="znine")
        nc.sync.dma_start(e00tmp[:], p2[6][12:13, :])
        nc.sync.dma_start(znine[:], zeros[:F + 1, :H * F])
        with nc.allow_low_precision(reason="fp32r exp rounding intended"):
            nc.vector.tensor_mul(znine[0:1, :], e00tmp[:], e00_t[:])
        nc.sync.dma_start(p2[6][12:12 + F + 1, :], znine[:])

        # l2 partial [1,108] via ones-matmul over token blocks (fp32)
        l2 = small.tile([1, H * F], FP32, tag="l2", name="l2")
        ps_l2 = psst.tile([1, 512], FP32, tag="stat", name="stat")
        for tb in range(NTB):
            p0, p1_ = tb * 128, min((tb + 1) * 128, T)
            nc.tensor.matmul(ps_l2[:, :H * F], ones_t[:p1_ - p0, :],
                             p2[tb][:p1_ - p0, :],
                             start=(tb == 0), stop=(tb == NTB - 1))
        nc.scalar.copy(l2[:], ps_l2[:, :H * F])

        # O2 partial [9, 768]: per head accumulate over token blocks
        o2 = small.tile([F, C], FP32, tag="o2", name="o2")
        for h in range(H):
            ps = pso2.tile([F, 64], FP32, tag="o2", name="o2")
            for tb in range(NTB):
                p0, p1_ = tb * 128, min((tb + 1) * 128, T)
                nc.tensor.matmul(ps[:, :],
                                 p2[tb][:p1_ - p0, h * F:(h + 1) * F],
                                 v_t[tb][:p1_ - p0, h * 64:(h + 1) * 64],
                                 start=(tb == 0), stop=(tb == NTB - 1))
            nc.scalar.copy(o2[:, h * 64:(h + 1) * 64], ps[:])

        # ---- pairwise AllReduce of (l2, o2)
        cc_in1 = dram.tile([F, C], FP32, tag="cc_in1", name="cc_in1")
        cc_out1 = dram.tile([F, C], FP32, tag="cc_out1", name="cc_out1")
        cc_in2 = dram.tile([1, H * F], FP32, tag="cc_in2", name="cc_in2")
        cc_out2 = dram.tile([1, H * F], FP32, tag="cc_out2", name="cc_out2")
        groups = [[0, 1], [2, 3], [4, 5], [6, 7]]
        nc.sync.dma_start(cc_in1[:], o2[:])
        nc.sync.dma_start(cc_in2[:], l2[:])
        nc.gpsimd.collective_compute("AllReduce", mybir.AluOpType.add,
                                     replica_groups=groups,
                                     ins=[cc_in1.opt()], outs=[cc_out1.opt()])
        nc.gpsimd.collective_compute("AllReduce", mybir.AluOpType.add,
                                     replica_groups=groups,
                                     ins=[cc_in2.opt()], outs=[cc_out2.opt()])
        o2m = small.tile([F, C], FP32, tag="o2m", name="o2m")
        l2m = small.tile([1, H * F], FP32, tag="l2m", name="l2m")
        nc.sync.dma_start(o2m[:], cc_out1[:])
        nc.sync.dma_start(l2m[:], cc_out2[:])

        # normalize: o2m[j, (h,d)] /= l2m[h*9+j]; build rl2 [9,12] token-major
        rl2 = small.tile([1, H * F], FP32, tag="rl2", name="rl2")
        nc.vector.reciprocal(rl2[:], l2m[:])
        rl2jh = small.tile([F, H], FP32, tag="rl2jh", name="rl2jh")
        # DMA remap [1,(h,j)] -> [j, h]: per j row, gather h with stride F
        for j in range(F):
            nc.sync.dma_start(rl2jh[j:j + 1, :], rl2[:, j::F])
        o2n = small.tile([F, C], FP32, tag="o2n", name="o2n")
        for h in range(H):
            nc.vector.tensor_scalar_mul(o2n[:, h * 64:(h + 1) * 64],
                                        o2m[:, h * 64:(h + 1) * 64],
                                        rl2jh[:, h:h + 1])

        # transpose [9, 768] -> attnout cols 780..789 (6 PE transposes)
        for ci in range(NCH):
            pst = psmm.tile([128, 512], FP32, tag="mm", name="mm")
            nc.tensor.transpose(pst[:128, :F], o2n[:, ci * 128:(ci + 1) * 128],
                                id_t[:F, :F])
            nc.scalar.copy(attnout[ci][:, SPH:SPH + F], pst[:128, :F])

        # =========================================================
        # STAGE C: proj (+bias) -> projout
        # =========================================================
        projout = [act.tile([128, T], FP32R, tag=f"h{ci}", name=f"h{ci}") for ci in range(NCH)]  # reuse h
        for cg in range(0, C, 512):
            gw = min(512, C - cg)
            wts = [wpool.tile([128, 512], BF16, tag="w", name="w") for _ in range(NCH)]
            for ci in range(NCH):
                nc.sync.dma_start(wts[ci][:, :gw],
                                  projWt[ci * 128:(ci + 1) * 128, cg:cg + gw])
            for co in range(gw // 128):
                cout = cg + co * 128
                for (t0, t1) in T_TILES:
                    ps = psmm.tile([128, 512], FP32, tag="mm", name="mm")
                    for ci in range(NCH):
                        nc.tensor.matmul(
                            ps[:, :t1 - t0],
                            wts[ci][:, co * 128:(co + 1) * 128],
                            attnout[ci][:, t0:t1],
                            start=(ci == 0), stop=(ci == NCH - 1))
                    nc.scalar.activation(projout[cout // 128][:, t0:t1],
                                         ps[:, :t1 - t0],
                                         mybir.ActivationFunctionType.Identity,
                                         bias=pb_t[cout // 128][:, 0:1])

        # =========================================================
        # STAGE D: LN2 + MLP + residual
        # =========================================================
        h2 = [act.tile([128, T], BF16, tag=f"q{ci}", name=f"q{ci}") for ci in range(NCH)]  # reuse q
        bc_a2 = small.tile([128, T], FP32, tag="bca", name="bca2")
        bc_b2 = small.tile([128, T], FP32, tag="bcb", name="bcb2")
        layer_norm_fm(projout, h2, "k", bc_a2, bc_b2)  # scratch shares k slots

        hid = [big.tile([128, T], BF16, tag=f"hid{ci}", name=f"hid{ci}") for ci in range(NCH_FF)]
        # hid tiles [128, T] x24 = 75.8KB/partition; shares 'big' v slots via tags?
        # (v is [128, C] x7 = 21.5KB; keep both tags distinct: v dead after O2 but
        #  tile pool tags differ in shape; rely on pool bufs=1 per tag.)

        # fc1 + gelu
        for cg in range(0, DFF, 512):
            wts = [wpool.tile([128, 512], BF16, tag="w", name="w") for _ in range(NCH)]
            for ci in range(NCH):
                nc.sync.dma_start(wts[ci][:],
                                  fc1Wt[ci * 128:(ci + 1) * 128, cg:cg + 512])
            for co in range(4):
                cout = cg + co * 128
                for (t0, t1) in T_TILES:
                    ps = psmm.tile([128, 512], FP32, tag="mm", name="mm")
                    for ci in range(NCH):
                        nc.tensor.matmul(
                            ps[:, :t1 - t0],
                            wts[ci][:, co * 128:(co + 1) * 128],
                            h2[ci][:, t0:t1],
                            start=(ci == 0), stop=(ci == NCH - 1))
                    nc.scalar.activation(hid[cout // 128][:, t0:t1],
                                         ps[:, :t1 - t0],
                                         mybir.ActivationFunctionType.Gelu,
                                         bias=f1b_t[cout // 128][:, 0:1])

        # fc2 + bias + residual -> DMA out
        for cb in range(NCH):           # output 128-blocks
            wts = [wpool2.tile([128, 128], BF16, tag="w2", name="w2") for _ in range(NCH_FF)]
            for ci in range(NCH_FF):
                nc.sync.dma_start(wts[ci][:],
                                  fc2Wt[ci * 128:(ci + 1) * 128,
                                        cb * 128:(cb + 1) * 128])
            for (t0, t1) in T_TILES:
                ps = psmm.tile([128, 512], FP32, tag="mm", name="mm")
                for ci in range(NCH_FF):
                    nc.tensor.matmul(ps[:, :t1 - t0], wts[ci][:],
                                     hid[ci][:, t0:t1],
                                     start=(ci == 0), stop=(ci == NCH_FF - 1))
                st = stage.tile([128, 512], FP32, tag="out", name="out")
                nc.scalar.activation(st[:, :t1 - t0], ps[:, :t1 - t0],
                                     mybir.ActivationFunctionType.Identity,
                                     bias=f2b_t[cb][:, 0:1])
                nc.vector.tensor_add(st[:, :t1 - t0], st[:, :t1 - t0],
                                     projout[cb][:, t0:t1])
                nc.sync.dma_start(outT[cb * 128:(cb + 1) * 128, t0:t1],
                                  st[:, :t1 - t0])

    nc.compile()
    return nc


# ---------------- host side ----------------
_compiled = {}


def kernel(**inputs):
    x = np.ascontiguousarray(np.asarray(inputs['x'], np.float32))
    qkv_w = np.asarray(inputs['qkv_w'], np.float32)
    proj_w = np.asarray(inputs['proj_w'], np.float32)
    proj_b = np.asarray(inputs['proj_b'], np.float32)
    fc1_w = np.asarray(inputs['fc1_w'], np.float32)
    fc1_b = np.asarray(inputs['fc1_b'], np.float32)
    fc2_w = np.asarray(inputs['fc2_w'], np.float32)
    fc2_b = np.asarray(inputs['fc2_b'], np.float32)
    g = np.asarray(inputs['ln2_g'], np.float32)
    bb = np.asarray(inputs['ln2_b'], np.float32)

    import ml_dtypes
    bf16 = ml_dtypes.bfloat16
    qkvWt = np.ascontiguousarray(qkv_w.T).astype(bf16)    # [768, 2304]
    projWt = np.ascontiguousarray(proj_w.T).astype(bf16)  # [768, 768]
    fc1Wt = np.ascontiguousarray(fc1_w.T).astype(bf16)    # [768, 3072]
    fc2Wt = np.ascontiguousarray(fc2_w.T).astype(bf16)    # [3072, 768]
    gb = np.ascontiguousarray(np.stack([g, bb], 1))          # [768, 2]
    ones = np.ones((128, 1), np.float32)
    onesrow_np = np.ones((1, 128), np.float32)
    headsel_np = np.zeros((H, C), np.float32)
    for h in range(H):
        headsel_np[h, h * 64:(h + 1) * 64] = 1.0
    bd9 = np.zeros((H * F, H), np.float32)
    for h in range(H):
        bd9[h * F:(h + 1) * F, h] = 1.0
    ident = np.eye(128, dtype=np.float32)
    e00_even = np.zeros((1, H * F), np.float32)
    e00_even[0, ::F] = 1.0
    e00_odd = np.zeros((1, H * F), np.float32)

    in_maps = []
    for core in range(8):
        b, half = core // 2, core % 2
        sp = x[b, F + half * SPH: F + (half + 1) * SPH]     # [780, C]
        tmp = x[b, 0:F]                                      # [9, C]
        pad = np.zeros((1, C), np.float32)
        xT = np.ascontiguousarray(np.concatenate([sp, tmp, pad], 0).T)  # [C, 790]
        in_maps.append(dict(
            xT=xT, qkvWt=qkvWt, projWt=projWt, fc1Wt=fc1Wt, fc2Wt=fc2Wt,
            gb=gb, projB=proj_b.reshape(C, 1), fc1B=fc1_b.reshape(DFF, 1),
            fc2B=fc2_b.reshape(C, 1), ones=ones, onesrow=onesrow_np,
            zeros=np.zeros((128, C), np.float32),
            headsel=headsel_np, bd9=bd9, ident=ident,
            e00mask=(e00_even if half == 0 else e00_odd)))

    if 'nc' not in _compiled:
        _compiled['nc'] = build_kernel()
    nc = _compiled['nc']
    res = run_bass_kernel_spmd(nc, in_maps, list(range(8)))
    _compiled['last_result'] = res

    out = np.zeros((B, N, C), np.float32)
    for core in range(8):
        b, half = core // 2, core % 2
        oT = res.results[core]['outT']                       # [C, 789]
        if half == 0:
            out[b, 0:F] = oT[:, SPH:SPH + F].T
            out[b, F:F + SPH] = oT[:, 0:SPH].T
        else:
            out[b, F + SPH:N] = oT[:, 0:SPH].T
    return out


if __name__ == '__main__':
    from reference import setup_inputs, reference
    inputs = {k: np.asarray(v) for k, v in setup_inputs().items()}
    out = kernel(**inputs)
    print("kernel ran, out shape", out.shape)



# revision 13
# speedup vs baseline: 1.8054x; 1.8054x over previous
"""Trainium2 Bass kernel for the sparse-attention ('interact' mask) transformer block.

Reference (B=4, N=1569, C=768, H=12, d=64, Dff=3072, F=9):
    h = LN(x)*g+b; qkv = h @ qkv_w.T; sparse attention (spatial rows attend
    only to the 9 temporal tokens, temporal rows attend to the 1560 spatial
    tokens, CLS also to itself); out = attn @ proj_w.T + proj_b;
    return out + MLP(LN(out)*g+b)

Sharding: 8 cores = 4 batches x 2 halves; local layout [780 spatial | 9
temporal | 1 zero pad] = 790 cols, feature-major [C, T] on chip.  Only
communication: pairwise AllReduce(add) of flash-style partials (l2, O2) for
the 9 temporal query rows, overlapped with the MLP GEMMs.

v2 design (vs. 479us baseline):
  - ALL matmul moving operands are bf16 (fp32r emitted fp32_mode=HIGH
    multi-pass: ~3x slower; measured).  LN g/b folded into weights/biases
    on the host so the device computes raw LN.
  - 2 token tiles (512+278) instead of 3 (512+268+10): the 10-col tile
    cost ~200ns/MM x 396 MMs = 83us of pure overhead.
  - weights resident/prefetched in an SBUF slot ring (no DMA serialization).
  - fc1+fc2 fused per hidden chunk (fc2 accumulates in 6 pinned PSUM banks)
    so no [128,T]x24 hid buffer and a dense warm PE stream.
  - second token tile of proj/LN2/MLP deferred behind the collective.
"""

import numpy as np
import sys
from contextlib import ExitStack

sys.path.insert(0, '/opt/trn_rl_repo')

import concourse.bass as bass
import concourse.bacc as bacc
import concourse.tile as tile
from concourse import mybir
from concourse.bass_utils import run_bass_kernel_spmd

# ---------------- problem constants ----------------
B, N, C = 4, 1569, 768
H, D = 12, 64
F = 9
DFF = 4 * C
NSP = N - F
SPH = NSP // 2
T = SPH + F + 1          # 790 local cols: [780 spatial | 9 temporal | 1 pad]
NCH = C // 128           # 6
NFF = DFF // 128         # 24
HF = H * F               # 108
NTB = (T + 127) // 128   # 7 token blocks (last = 22 rows)
SCALE = D ** -0.5

FP32 = mybir.dt.float32
BF16 = mybir.dt.bfloat16
AF = mybir.ActivationFunctionType
OP = mybir.AluOpType

TT = [(0, 512), (512, T)]        # main 2-tile split
TSP = [(0, 512), (512, SPH)]     # spatial-only (O1)


def build_kernel():
    nc = bacc.Bacc("TRN2", target_bir_lowering=False, debug=False,
                   num_devices=8)

    # ---------------- DRAM I/O ----------------
    xT = nc.dram_tensor("xT", [C, T], FP32, kind="ExternalInput")
    qkvWt = nc.dram_tensor("qkvWt", [C, 3 * C], BF16, kind="ExternalInput")
    projWt = nc.dram_tensor("projWt", [C, C], BF16, kind="ExternalInput")
    fc1Wt = nc.dram_tensor("fc1Wt", [C, DFF], BF16, kind="ExternalInput")
    fc2Wt = nc.dram_tensor("fc2Wt", [DFF, C], BF16, kind="ExternalInput")
    qkvB = nc.dram_tensor("qkvB", [2 * C, 1], FP32, kind="ExternalInput")
    vbrow = nc.dram_tensor("vbrow", [1, C], BF16, kind="ExternalInput")
    projB = nc.dram_tensor("projB", [C, 1], FP32, kind="ExternalInput")
    fc1B = nc.dram_tensor("fc1B", [DFF, 1], FP32, kind="ExternalInput")
    fc2B = nc.dram_tensor("fc2B", [C, 1], FP32, kind="ExternalInput")
    ones = nc.dram_tensor("ones", [128, 1], BF16, kind="ExternalInput")
    onesrow = nc.dram_tensor("onesrow", [1, 128], BF16, kind="ExternalInput")
    bd9 = nc.dram_tensor("bd9", [HF, H], BF16, kind="ExternalInput")
    bd9T = nc.dram_tensor("bd9T", [H, HF], BF16, kind="ExternalInput")
    ident = nc.dram_tensor("ident", [128, 128], BF16, kind="ExternalInput")
    e00 = nc.dram_tensor("e00", [128, HF], BF16, kind="ExternalInput")
    outT = nc.dram_tensor("outT", [C, T], FP32, kind="ExternalOutput")

    with tile.TileContext(nc) as tc, ExitStack() as ctx:
        cst = ctx.enter_context(tc.tile_pool(name="cst", bufs=1))
        wb = ctx.enter_context(tc.tile_pool(name="wb", bufs=12))
        wp = ctx.enter_context(tc.tile_pool(name="wp", bufs=6))
        xfp = ctx.enter_context(tc.tile_pool(name="xfp", bufs=6))
        aa = ctx.enter_context(tc.tile_pool(name="aa", bufs=18))
        sep = ctx.enter_context(tc.tile_pool(name="sep", bufs=1))
        sm = ctx.enter_context(tc.tile_pool(name="sm", bufs=1))
        ps = ctx.enter_context(tc.tile_pool(name="ps", bufs=1, space="PSUM"))
        dram = ctx.enter_context(tc.tile_pool(name="dram", bufs=1, space="DRAM"))

        # psum helper: rotate through mm(2) + acc0..5(1) for general groups
        ps_cycle = ['mm', 'acc0', 'mm', 'acc1', 'mm', 'acc2',
                    'mm', 'acc3', 'mm', 'acc4', 'mm', 'acc5']
        ps_i = [0]

        def pt(tag=None):
            if tag is None:
                tag = ps_cycle[ps_i[0] % len(ps_cycle)]
                ps_i[0] += 1
            return ps.tile([128, 512], FP32, tag=tag,
                           bufs=(2 if tag == 'mm' else 1), name="pst")

        # ---------------- constants ----------------
        ones_t = cst.tile([128, 1], BF16, tag="ones", name="ones")
        nc.sync.dma_start(ones_t[:], ones[:])
        onesrow_t = cst.tile([1, 128], BF16, tag="onesrow", name="onesrow")
        nc.sync.dma_start(onesrow_t[:], onesrow[:])
        bd9_t = cst.tile([HF, H], BF16, tag="bd9", name="bd9")
        nc.sync.dma_start(bd9_t[:], bd9[:])
        bd9T_t = cst.tile([H, HF], BF16, tag="bd9T", name="bd9T")
        nc.sync.dma_start(bd9T_t[:], bd9T[:])
        id_t = cst.tile([128, 128], BF16, tag="ident", name="ident")
        nc.sync.dma_start(id_t[:], ident[:])
        e00_t = cst.tile([128, HF], BF16, tag="e00", name="e00")
        nc.sync.dma_start(e00_t[:], e00[:])
        vbrow_t = cst.tile([1, C], BF16, tag="vbrow", name="vbrow")
        nc.sync.dma_start(vbrow_t[:], vbrow[:])
        qb_t = [cst.tile([128, 1], FP32, tag=f"qb{j}", name=f"qb{j}")
                for j in range(12)]
        for j in range(12):
            nc.sync.dma_start(qb_t[j][:], qkvB[j * 128:(j + 1) * 128, :])
        pb_t = [cst.tile([128, 1], FP32, tag=f"pb{j}", name=f"pb{j}")
                for j in range(NCH)]
        for j in range(NCH):
            nc.sync.dma_start(pb_t[j][:], projB[j * 128:(j + 1) * 128, :])
        f1b_t = [cst.tile([128, 1], FP32, tag=f"f1b{j}", name=f"f1b{j}")
                 for j in range(NFF)]
        for j in range(NFF):
            nc.sync.dma_start(f1b_t[j][:], fc1B[j * 128:(j + 1) * 128, :])
        f2b_t = [cst.tile([128, 1], FP32, tag=f"f2b{j}", name=f"f2b{j}")
                 for j in range(NCH)]
        for j in range(NCH):
            nc.sync.dma_start(f2b_t[j][:], fc2B[j * 128:(j + 1) * 128, :])

        # ---------------- x load (first on the sync queue) ----------------
        xf = [xfp.tile([128, T], FP32, tag="xf", name=f"xf{ci}")
              for ci in range(NCH)]
        for ci in range(NCH):
            nc.sync.dma_start(xf[ci][:], xT[ci * 128:(ci + 1) * 128, :])

        # ---------------- weight prefetch ----------------
        # qkv q/k: 6 x [128,2304] in the big ring, column-group-ordered DMAs
        wqkv = [wb.tile([128, 3 * C], BF16, tag="w", name=f"wqkv{ci}",
                        padded_shape=[128, DFF]) for ci in range(NCH)]
        for cg in range(0, 3 * C, 512):
            gw = min(512, 3 * C - cg)
            for ci in range(NCH):
                nc.sync.dma_start(wqkv[ci][:, cg:cg + gw],
                                  qkvWt[ci * 128:(ci + 1) * 128, cg:cg + gw])
        wpj = [wp.tile([128, C], BF16, tag="wp", name=f"wpj{ci}")
               for ci in range(NCH)]
        for ci in range(NCH):
            nc.sync.dma_start(wpj[ci][:], projWt[ci * 128:(ci + 1) * 128, :])
        wf1 = [wb.tile([128, DFF], BF16, tag="w", name=f"wf1{ci}")
               for ci in range(NCH)]
        for ci in range(NCH):
            nc.sync.dma_start(wf1[ci][:], fc1Wt[ci * 128:(ci + 1) * 128, :])

        # ---------------- casts for LN1 stats ----------------
        xb = [aa.tile([128, T], BF16, tag="a", name=f"xb{ci}")
              for ci in range(NCH)]
        sq = [aa.tile([128, T], BF16, tag="a", name=f"sq{ci}")
              for ci in range(NCH)]
        for ci in range(NCH):
            nc.vector.tensor_copy(xb[ci][:], xf[ci][:])
            nc.scalar.activation(sq[ci][:], xf[ci][:], AF.Square)

        # ---------------- LN helpers ----------------
        def ln_stats(srcb, srcsq, t0, t1):
            w = t1 - t0
            p = pt()
            for ci in range(NCH):
                nc.tensor.matmul(p[0:1, :w], ones_t[:, 0:1],
                                 srcb[ci][:, t0:t1],
                                 start=(ci == 0), stop=(ci == NCH - 1))
            p2 = pt()
            for ci in range(NCH):
                nc.tensor.matmul(p2[0:1, :w], ones_t[:, 0:1],
                                 srcsq[ci][:, t0:t1],
                                 start=(ci == 0), stop=(ci == NCH - 1))
            return p, p2

        def ln_chain(p, p2, t0, t1):
            w = t1 - t0
            mu = sm.tile([1, 512], FP32, tag="mu", name="mu", bufs=1)
            tmp = sm.tile([1, 512], FP32, tag="tmp", name="tmp", bufs=1)
            al = sm.tile([1, 512], BF16, tag="al", name="al", bufs=1)
            be = sm.tile([1, 512], BF16, tag="be", name="be", bufs=1)
            nc.vector.tensor_scalar_mul(mu[0:1, :w], p[0:1, :w], 1.0 / C)
            nc.vector.tensor_mul(tmp[0:1, :w], mu[0:1, :w], mu[0:1, :w])
            # tmp = mu^2 - eps  so that  var+eps = ps2/C - tmp
            nc.vector.tensor_scalar_add(tmp[0:1, :w], tmp[0:1, :w], -1e-5)
            nc.vector.scalar_tensor_tensor(p2[0:1, :w], p2[0:1, :w], 1.0 / C,
                                           tmp[0:1, :w],
                                           op0=OP.mult, op1=OP.subtract)
            nc.scalar.activation(tmp[0:1, :w], p2[0:1, :w], AF.Sqrt)
            with nc.allow_low_precision(reason="bf16 LN scale intended"):
                nc.vector.reciprocal(al[0:1, :w], tmp[0:1, :w])
                nc.vector.scalar_tensor_tensor(be[0:1, :w], mu[0:1, :w],
                                               -1.0, al[0:1, :w],
                                               op0=OP.mult, op1=OP.mult)
            return al, be

        def ln_bcast(al, be, bcA, bcB, t0, t1):
            w = t1 - t0
            for src, dst in ((al, bcA), (be, bcB)):
                psb = pt(tag='mm')
                nc.tensor.matmul(psb[:, :w], onesrow_t[0:1, :],
                                 src[0:1, :w], start=True, stop=True)
                nc.scalar.copy(dst[:, t0:t1], psb[:, :w])

        def ln_apply(srcf, dst, bcA, bcB, t0, t1):
            for ci in range(NCH):
                nc.vector.tensor_mul(dst[ci][:, t0:t1], srcf[ci][:, t0:t1],
                                     bcA[:, t0:t1])
                nc.vector.tensor_tensor(dst[ci][:, t0:t1], dst[ci][:, t0:t1],
                                        bcB[:, t0:t1], op=OP.add)

        # ---------------- LN1 ----------------
        h = [aa.tile([128, T], BF16, tag="a", name=f"h{ci}")
             for ci in range(NCH)]
        bcA1 = sm.tile([128, T], BF16, tag="bc", name="bcA1", bufs=2)
        bcB1 = sm.tile([128, T], BF16, tag="bc", name="bcB1", bufs=2)

        s_t0, s2_t0 = ln_stats(xb, sq, *TT[0])
        al0, be0 = ln_chain(s_t0, s2_t0, *TT[0])
        ln_bcast(al0, be0, bcA1, bcB1, *TT[0])
        ln_apply(xf, h, bcA1, bcB1, *TT[0])
        s_t1, s2_t1 = ln_stats(xb, sq, *TT[1])
        al1, be1 = ln_chain(s_t1, s2_t1, *TT[1])

        # ---------------- qkv (q,k feature-major) ----------------
        k_t = [aa.tile([128, T], BF16, tag="a", name=f"k{ci}")
               for ci in range(NCH)]
        q_t = [aa.tile([128, T], BF16, tag="a", name=f"q{ci}")
               for ci in range(NCH)]

        def qk_group(j, t0, t1):
            # j in 0..11: 0-5 = q couts, 6-11 = k couts
            w = t1 - t0
            dst = q_t[j] if j < NCH else k_t[j - NCH]
            p = pt()
            for ci in range(NCH):
                nc.tensor.matmul(p[:, :w], wqkv[ci][:, j * 128:(j + 1) * 128],
                                 h[ci][:, t0:t1],
                                 start=(ci == 0), stop=(ci == NCH - 1))
            nc.scalar.activation(dst[:, t0:t1], p[:, :w], AF.Identity,
                                 bias=qb_t[j][:, 0:1])

        for j in range(3):
            qk_group(j, *TT[0])
        # t1 broadcast + apply overlap the first qkv groups
        ln_bcast(al1, be1, bcA1, bcB1, *TT[1])
        ln_apply(xf, h, bcA1, bcB1, *TT[1])
        for j in range(3, 12):
            qk_group(j, *TT[0])
        for j in range(12):
            qk_group(j, *TT[1])

        # v bias broadcast [128, C]
        vb_bc = sm.tile([128, C], BF16, tag="vbbc", name="vb_bc")
        for cg in range(0, C, 512):
            gw = min(512, C - cg)
            psb = pt()
            nc.tensor.matmul(psb[:, :gw], onesrow_t[0:1, :],
                             vbrow_t[0:1, cg:cg + gw], start=True, stop=True)
            nc.scalar.copy(vb_bc[:, cg:cg + gw], psb[:, :gw])

        # v token-major [T, C]
        v_t = [sep.tile([128, C], BF16, tag="v", name=f"v{tb}", bufs=NTB)
               for tb in range(NTB)]
        for tb in range(NTB):
            p0, p1_ = tb * 128, min((tb + 1) * 128, T)
            pp = p1_ - p0
            for cg in range(0, C, 512):
                gw = min(512, C - cg)
                p = pt()
                for ci in range(NCH):
                    nc.tensor.matmul(p[:pp, :gw], h[ci][:, p0:p1_],
                                     wqkv[ci][:, 2 * C + cg:2 * C + cg + gw],
                                     start=(ci == 0), stop=(ci == NCH - 1))
                nc.vector.tensor_tensor(v_t[tb][:pp, cg:cg + gw], p[:pp, :gw],
                                        vb_bc[:pp, cg:cg + gw], op=OP.add)

        # =========================================================
        # sparse attention
        # =========================================================
        attnout = [sep.tile([128, T], BF16, tag="ao", name=f"ao{ci}", bufs=6)
                   for ci in range(NCH)]
        for ci in range(NCH):
            # zero pad col 789 (and 788, rewritten by the temporal patch)
            nc.vector.memzero(attnout[ci][:, T - 2:T])

        # kbd (h,j) cols / qbd (j,h) cols, block-diag by head
        kbd = [sm.tile([128, HF], BF16, tag=f"kbd{ci}", name=f"kbd{ci}")
               for ci in range(NCH)]
        qbd = [sm.tile([128, HF], BF16, tag=f"qbd{ci}", name=f"qbd{ci}")
               for ci in range(NCH)]
        for ci in range(NCH):
            nc.vector.memzero(kbd[ci][:])
            nc.vector.memzero(qbd[ci][:])
        for hh in range(H):
            ci, po = hh // 2, (hh % 2) * 64
            nc.vector.tensor_copy(kbd[ci][po:po + 64, hh * F:(hh + 1) * F],
                                  k_t[ci][po:po + 64, SPH:SPH + F])
            nc.vector.tensor_copy(qbd[ci][po:po + 64, hh:hh + 97:H],
                                  q_t[ci][po:po + 64, SPH:SPH + F])

        # vtmp_bd [108, C]: rows (h,j) = temporal v of head h at cols h*64..
        vtmp_bd = sm.tile([HF, C], BF16, tag="vtmpbd", name="vtmpbd")
        nc.vector.memzero(vtmp_bd[0:HF, :])
        for hh in range(H):
            nc.scalar.dma_start(vtmp_bd[hh * F:(hh + 1) * F,
                                        hh * 64:(hh + 1) * 64],
                                v_t[6][12:12 + F, hh * 64:(hh + 1) * 64])

        # S1/P1: all local queries vs 9 temporal keys -> p1 [108, T]
        p1 = sm.tile([HF, T], BF16, tag="p1", name="p1")
        for (t0, t1) in TT:
            w = t1 - t0
            p = pt()
            for ci in range(NCH):
                nc.tensor.matmul(p[0:HF, :w], kbd[ci][:], q_t[ci][:, t0:t1],
                                 start=(ci == 0), stop=(ci == NCH - 1))
            nc.scalar.activation(p1[0:HF, t0:t1], p[0:HF, :w], AF.Exp,
                                 scale=SCALE)

        # lsp[h,t] = sum_j p1[(h,j),t]; rlsp = 1/lsp (bf16)
        rlsp = sm.tile([H, T], BF16, tag="rlsp", name="rlsp")
        for (t0, t1) in TT:
            w = t1 - t0
            p = pt()
            nc.tensor.matmul(p[0:H, :w], bd9_t[0:HF, :], p1[0:HF, t0:t1],
                             start=True, stop=True)
            with nc.allow_low_precision(reason="bf16 softmax recip intended"):
                nc.vector.reciprocal(rlsp[0:H, t0:t1], p[0:H, :w])

        # rlsp9 [108, T] = rlsp repeated per j; p1 *= rlsp9 (pre-normalize)
        rlsp9 = sm.tile([HF, T], BF16, tag="rlsp9", name="rlsp9")
        for (t0, t1) in TT:
            w = t1 - t0
            p = pt()
            nc.tensor.matmul(p[0:HF, :w], bd9T_t[0:H, :], rlsp[0:H, t0:t1],
                             start=True, stop=True)
            nc.vector.tensor_copy(rlsp9[0:HF, t0:t1], p[0:HF, :w])
        nc.vector.tensor_mul(p1[0:HF, :], p1[0:HF, :], rlsp9[0:HF, :])

        # O1: spatial attention out
        for ci in range(NCH):
            for (t0, t1) in TSP:
                w = t1 - t0
                p = pt()
                nc.tensor.matmul(p[:, :w],
                                 vtmp_bd[0:HF, ci * 128:(ci + 1) * 128],
                                 p1[0:HF, t0:t1], start=True, stop=True)
                nc.vector.tensor_copy(attnout[ci][:, t0:t1], p[:, :w])

        # S2T/P2T: temporal queries vs all local keys, token-major [T, 108]
        p2 = [sm.tile([128, HF], BF16, tag="p2", name=f"p2{tb}", bufs=NTB)
              for tb in range(NTB)]
        for tb in range(NTB):
            p0, p1_ = tb * 128, min((tb + 1) * 128, T)
            pp = p1_ - p0
            p = pt()
            for ci in range(NCH):
                nc.tensor.matmul(p[:pp, 0:HF], k_t[ci][:, p0:p1_], qbd[ci][:],
                                 start=(ci == 0), stop=(ci == NCH - 1))
            nc.scalar.activation(p2[tb][:pp, :], p[:pp, 0:HF], AF.Exp,
                                 scale=SCALE)
        # mask token-block 6: rows 0-11 (spatial) pass, row 12 (CLS key)
        # kept only for q_j=0 on even cores, rows 13-21 (temporal+pad) zeroed
        nc.vector.tensor_mul(p2[6][0:22, :], p2[6][0:22, :], e00_t[0:22, :])

        # l2 partial [1,108]
        l2row = sm.tile([1, HF], FP32, tag="l2", name="l2row")
        p_l2 = pt()
        for tb in range(NTB):
            p0, p1_ = tb * 128, min((tb + 1) * 128, T)
            pp = p1_ - p0
            nc.tensor.matmul(p_l2[0:1, 0:HF], ones_t[:pp, 0:1],
                             p2[tb][:pp, :],
                             start=(tb == 0), stop=(tb == NTB - 1))
        nc.scalar.copy(l2row[:], p_l2[0:1, 0:HF])

        # O2 partial [9, C]
        o2 = sm.tile([F, C], FP32, tag="o2", name="o2")
        for hh in range(H):
            p = pt()
            for tb in range(NTB):
                p0, p1_ = tb * 128, min((tb + 1) * 128, T)
                pp = p1_ - p0
                nc.tensor.matmul(p[0:F, 0:64],
                                 p2[tb][:pp, hh:hh + 97:H],
                                 v_t[tb][:pp, hh * 64:(hh + 1) * 64],
                                 start=(tb == 0), stop=(tb == NTB - 1))
            nc.scalar.copy(o2[0:F, hh * 64:(hh + 1) * 64], p[0:F, 0:64])

        # pairwise AllReduce of (o2 | l2) in one [10, C] buffer
        cc_in = dram.tile([F + 1, C], FP32, tag="cc_in", name="cc_in")
        cc_out = dram.tile([F + 1, C], FP32, tag="cc_out", name="cc_out")
        groups = [[0, 1], [2, 3], [4, 5], [6, 7]]
        nc.scalar.dma_start(cc_in[0:F, :], o2[0:F, :])
        nc.scalar.dma_start(cc_in[F:F + 1, 0:HF], l2row[:])
        nc.gpsimd.collective_compute("AllReduce", mybir.AluOpType.add,
                                     replica_groups=groups,
                                     ins=[cc_in.opt()], outs=[cc_out.opt()])

        # ---------------- fc2 weights into recycled qkv slots ----------------
        w2g = [wb.tile([128, DFF], BF16, tag="w", name=f"w2g{g}")
               for g in range(NCH)]
        for g in range(NCH):
            for kk in range(4):
                cchunk = 4 * g + kk
                nc.sync.dma_start(w2g[g][:, kk * C:(kk + 1) * C],
                                  fc2Wt[cchunk * 128:(cchunk + 1) * 128, :])

        # =========================================================
        # proj t0 -> LN2 t0 -> fused fc1+fc2 t0 (collective overlapped)
        # =========================================================
        projout = [xfp.tile([128, T], FP32, tag="xf", name=f"po{ci}")
                   for ci in range(NCH)]
        pb = [sep.tile([128, T], BF16, tag="pbb", name=f"pbb{ci}", bufs=6)
              for ci in range(NCH)]
        sq2 = [aa.tile([128, T], BF16, tag="a", name=f"sq2{ci}")
               for ci in range(NCH)]
        h2 = [sep.tile([128, T], BF16, tag="h2", name=f"h2{ci}", bufs=6)
              for ci in range(NCH)]
        bcA2 = sm.tile([128, T], BF16, tag="bc", name="bcA2", bufs=2)
        bcB2 = sm.tile([128, T], BF16, tag="bc", name="bcB2", bufs=2)

        def proj_tile(t0, t1):
            w = t1 - t0
            for j in range(NCH):
                p = pt(tag='mm')
                for ci in range(NCH):
                    nc.tensor.matmul(p[:, :w],
                                     wpj[ci][:, j * 128:(j + 1) * 128],
                                     attnout[ci][:, t0:t1],
                                     start=(ci == 0), stop=(ci == NCH - 1))
                nc.scalar.activation(projout[j][:, t0:t1], p[:, :w],
                                     AF.Identity, bias=pb_t[j][:, 0:1])
                nc.scalar.activation(sq2[j][:, t0:t1], p[:, :w], AF.Square,
                                     bias=pb_t[j][:, 0:1])
                nc.vector.tensor_scalar_add(pb[j][:, t0:t1], p[:, :w],
                                            pb_t[j][:, 0:1])

        def ln2_tile(t0, t1):
            w = t1 - t0
            p = pt(tag='mm')
            for ci in range(NCH):
                nc.tensor.matmul(p[0:1, :w], ones_t[:, 0:1], pb[ci][:, t0:t1],
                                 start=(ci == 0), stop=(ci == NCH - 1))
            p2_ = pt(tag='mm')
            for ci in range(NCH):
                nc.tensor.matmul(p2_[0:1, :w], ones_t[:, 0:1],
                                 sq2[ci][:, t0:t1],
                                 start=(ci == 0), stop=(ci == NCH - 1))
            al2, be2 = ln_chain(p, p2_, t0, t1)
            ln_bcast(al2, be2, bcA2, bcB2, t0, t1)
            ln_apply(projout, h2, bcA2, bcB2, t0, t1)

        proj_tile(*TT[0])
        ln2_tile(*TT[0])

        # fused fc1+fc2: fc2 accumulates into 6 pinned PSUM banks
        def fused(t0, t1, mid_hook=None):
            w = t1 - t0
            acc = [pt(tag=f'acc{cb}') for cb in range(NCH)]
            for g in range(NFF):
                if mid_hook is not None and g == 12:
                    mid_hook()
                pf = pt(tag='mm')
                for ci in range(NCH):
                    nc.tensor.matmul(pf[:, :w],
                                     wf1[ci][:, g * 128:(g + 1) * 128],
                                     h2[ci][:, t0:t1],
                                     start=(ci == 0), stop=(ci == NCH - 1))
                hidt = sm.tile([128, 512], BF16, tag="hid", name=f"hid{g}",
                               bufs=4)
                nc.scalar.activation(hidt[:, :w], pf[:, :w], AF.Gelu,
                                     bias=f1b_t[g][:, 0:1])
                wg, kk = g // 4, g % 4
                for cb in range(NCH):
                    nc.tensor.matmul(acc[cb][:, :w],
                                     w2g[wg][:, kk * C + cb * 128:
                                             kk * C + (cb + 1) * 128],
                                     hidt[:, :w],
                                     start=(g == 0), stop=(g == NFF - 1),
                                     skip_group_check=True)
            for cb in range(NCH):
                st = sm.tile([128, 512], FP32, tag="st", name=f"st{cb}",
                             bufs=2)
                nc.vector.scalar_tensor_tensor(st[:, :w], acc[cb][:, :w],
                                               f2b_t[cb][:, 0:1],
                                               projout[cb][:, t0:t1],
                                               op0=OP.add, op1=OP.add)
                nc.sync.dma_start(outT[cb * 128:(cb + 1) * 128, t0:t1],
                                  st[:, :w])

        # deferred: collective landing -> temporal cols -> t1 of everything
        def temporal_patch():
            l2jh = sm.tile([F, H], FP32, tag="l2jh", name="l2jh")
            o2n = sm.tile([F, C], BF16, tag="o2n", name="o2n")
            # SWDGE cast-DMA f32 -> bf16 straight into o2n
            nc.gpsimd.dma_start(o2n[0:F, :], cc_out[0:F, :])
            for j in range(F):
                nc.scalar.dma_start(l2jh[j:j + 1, :],
                                    cc_out[F:F + 1, j * H:(j + 1) * H])
            nc.vector.reciprocal(l2jh[0:F, :], l2jh[0:F, :])
            for hh in range(H):
                nc.vector.tensor_scalar_mul(o2n[0:F, hh * 64:(hh + 1) * 64],
                                            o2n[0:F, hh * 64:(hh + 1) * 64],
                                            l2jh[0:F, hh:hh + 1])
            for ci in range(NCH):
                p = ps.tile([128, 512], BF16, tag='mm', bufs=2, name="pstb")
                nc.tensor.transpose(p[:, 0:F],
                                    o2n[0:F, ci * 128:(ci + 1) * 128],
                                    id_t[0:F, 0:F])
                nc.scalar.copy(attnout[ci][:, SPH:SPH + F], p[:, 0:F])
            proj_tile(*TT[1])
            ln2_tile(*TT[1])

        fused(*TT[0], mid_hook=temporal_patch)
        fused(*TT[1])

    nc.compile()
    return nc


# ---------------- host side ----------------
_compiled = {}


def kernel(**inputs):
    x = np.ascontiguousarray(np.asarray(inputs['x'], np.float32))
    qkv_w = np.asarray(inputs['qkv_w'], np.float32)
    proj_w = np.asarray(inputs['proj_w'], np.float32)
    proj_b = np.asarray(inputs['proj_b'], np.float32)
    fc1_w = np.asarray(inputs['fc1_w'], np.float32)
    fc1_b = np.asarray(inputs['fc1_b'], np.float32)
    fc2_w = np.asarray(inputs['fc2_w'], np.float32)
    fc2_b = np.asarray(inputs['fc2_b'], np.float32)
    g = np.asarray(inputs['ln2_g'], np.float32)
    bb = np.asarray(inputs['ln2_b'], np.float32)

    import ml_dtypes
    bf16 = ml_dtypes.bfloat16

    # fold LN affine (g, b) into the consuming GEMMs:
    #   W @ (LNraw(x)*g + b) = (W*g) @ LNraw(x) + W@b
    qkvW = qkv_w * g[None, :]                 # [3C, C]
    qkvB = qkv_w @ bb                         # [3C]
    fc1W = fc1_w * g[None, :]
    fc1Bf = fc1_b + fc1_w @ bb

    qkvWt = np.ascontiguousarray(qkvW.T).astype(bf16)     # [C, 3C]
    projWt = np.ascontiguousarray(proj_w.T).astype(bf16)  # [C, C]
    fc1Wt = np.ascontiguousarray(fc1W.T).astype(bf16)     # [C, DFF]
    fc2Wt = np.ascontiguousarray(fc2_w.T).astype(bf16)    # [DFF, C]

    ones_np = np.ones((128, 1), np.float32).astype(bf16)
    onesrow_np = np.ones((1, 128), np.float32).astype(bf16)
    bd9_np = np.zeros((H * F, H), np.float32)
    for hh in range(H):
        bd9_np[hh * F:(hh + 1) * F, hh] = 1.0
    bd9T_np = np.ascontiguousarray(bd9_np.T)
    ident_np = np.eye(128, dtype=np.float32).astype(bf16)
    # e00: multiplicative mask for p2 token-block 6 (local tokens 768..789):
    # rows 0-11 = spatial -> 1; row 12 = CLS key -> keep only q_j=0 cols
    # (cols 0..11 in (j,h) order) on even cores; rows 13-21 = temporal
    # non-CLS keys + pad -> 0
    e00_even = np.zeros((128, H * F), np.float32)
    e00_even[0:12, :] = 1.0
    e00_even[12, 0:H] = 1.0
    e00_odd = np.zeros((128, H * F), np.float32)
    e00_odd[0:12, :] = 1.0

    in_maps = []
    for core in range(8):
        b_, half = core // 2, core % 2
        sp = x[b_, F + half * SPH: F + (half + 1) * SPH]     # [780, C]
        tmp = x[b_, 0:F]                                     # [9, C]
        pad = np.zeros((1, C), np.float32)
        xTn = np.ascontiguousarray(
            np.concatenate([sp, tmp, pad], 0).T)             # [C, 790]
        in_maps.append(dict(
            xT=xTn, qkvWt=qkvWt, projWt=projWt, fc1Wt=fc1Wt, fc2Wt=fc2Wt,
            qkvB=qkvB[:2 * C].reshape(2 * C, 1),
            vbrow=qkvB[2 * C:].reshape(1, C).astype(bf16),
            projB=proj_b.reshape(C, 1), fc1B=fc1Bf.reshape(DFF, 1),
            fc2B=fc2_b.reshape(C, 1),
            ones=ones_np, onesrow=onesrow_np,
            bd9=bd9_np.astype(bf16), bd9T=bd9T_np.astype(bf16),
            ident=ident_np,
            e00=(e00_even if half == 0 else e00_odd).astype(bf16)))

    if 'nc' not in _compiled:
        _compiled['nc'] = build_kernel()
    nc = _compiled['nc']
    res = run_bass_kernel_spmd(nc, in_maps, list(range(8)))
    _compiled['last_result'] = res

    out = np.zeros((B, N, C), np.float32)
    for core in range(8):
        b_, half = core // 2, core % 2
        oT = res.results[core]['outT']                       # [C, 790]
        if half == 0:
            out[b_, 0:F] = oT[:, SPH:SPH + F].T
            out[b_, F:F + SPH] = oT[:, 0:SPH].T
        else:
            out[b_, F + SPH:N] = oT[:, 0:SPH].T
    return out


if __name__ == '__main__':
    from reference import setup_inputs, reference
    inputs = {k: np.asarray(v) for k, v in setup_inputs().items()}
    out = kernel(**inputs)
    print("kernel ran, out shape", out.shape)


# revision 19
# speedup vs baseline: 1.8506x; 1.0250x over previous
"""Trainium2 Bass kernel for the sparse-attention ('interact' mask) transformer block.

Reference (B=4, N=1569, C=768, H=12, d=64, Dff=3072, F=9):
    h = LN(x)*g+b; qkv = h @ qkv_w.T; sparse attention (spatial rows attend
    only to the 9 temporal tokens, temporal rows attend to the 1560 spatial
    tokens, CLS also to itself); out = attn @ proj_w.T + proj_b;
    return out + MLP(LN(out)*g+b)

Sharding: 8 cores = 4 batches x 2 halves; local layout [780 spatial | 9
temporal | 1 zero pad] = 790 cols, feature-major [C, T] on chip.  Only
communication: pairwise AllReduce(add) of flash-style partials (l2, O2) for
the 9 temporal query rows, overlapped with the MLP GEMMs.

v2 design (vs. 479us baseline):
  - ALL matmul moving operands are bf16 (fp32r emitted fp32_mode=HIGH
    multi-pass: ~3x slower; measured).  LN g/b folded into weights/biases
    on the host so the device computes raw LN.
  - 2 token tiles (512+278) instead of 3 (512+268+10): the 10-col tile
    cost ~200ns/MM x 396 MMs = 83us of pure overhead.
  - weights resident/prefetched in an SBUF slot ring (no DMA serialization).
  - fc1+fc2 fused per hidden chunk (fc2 accumulates in 6 pinned PSUM banks)
    so no [128,T]x24 hid buffer and a dense warm PE stream.
  - second token tile of proj/LN2/MLP deferred behind the collective.
"""

import numpy as np
import sys
from contextlib import ExitStack

sys.path.insert(0, '/opt/trn_rl_repo')

import concourse.bass as bass
import concourse.bacc as bacc
import concourse.tile as tile
from concourse import mybir
from concourse.bass_utils import run_bass_kernel_spmd

# ---------------- problem constants ----------------
B, N, C = 4, 1569, 768
H, D = 12, 64
F = 9
DFF = 4 * C
NSP = N - F
SPH = NSP // 2
T = SPH + F + 1          # 790 local cols: [780 spatial | 9 temporal | 1 pad]
NCH = C // 128           # 6
NFF = DFF // 128         # 24
HF = H * F               # 108
NTB = (T + 127) // 128   # 7 token blocks (last = 22 rows)
SCALE = D ** -0.5

FP32 = mybir.dt.float32
BF16 = mybir.dt.bfloat16
AF = mybir.ActivationFunctionType
OP = mybir.AluOpType

TT = [(0, 512), (512, T)]        # main 2-tile split
TSP = [(0, 512), (512, SPH)]     # spatial-only (O1)


def build_kernel():
    nc = bacc.Bacc("TRN2", target_bir_lowering=False, debug=False,
                   num_devices=8)

    # ---------------- DRAM I/O ----------------
    xT = nc.dram_tensor("xT", [C, T], FP32, kind="ExternalInput")
    qkvWt = nc.dram_tensor("qkvWt", [C, 3 * C], BF16, kind="ExternalInput")
    projWt = nc.dram_tensor("projWt", [C, C], BF16, kind="ExternalInput")
    fc1Wt = nc.dram_tensor("fc1Wt", [C, DFF], BF16, kind="ExternalInput")
    fc2Wt = nc.dram_tensor("fc2Wt", [DFF, C], BF16, kind="ExternalInput")
    # packed constants: auxf [128,48] fp32 = biases (qk 0:12 | proj 12:18 |
    # fc1 18:42 | fc2 42:48); auxb [128,1253] bf16 = ones(0) | ident(1:129) |
    # e00(129:237) | bd9(237:249) | onesrow(249:377) | bd9T(377:485) |
    # vbrow(485:1253)
    auxf = nc.dram_tensor("auxf", [128, 48], FP32, kind="ExternalInput")
    auxb = nc.dram_tensor("auxb", [128, 1253], BF16, kind="ExternalInput")
    outT = nc.dram_tensor("outT", [C, T], FP32, kind="ExternalOutput")

    with tile.TileContext(nc) as tc, ExitStack() as ctx:
        cst = ctx.enter_context(tc.tile_pool(name="cst", bufs=1))
        wb = ctx.enter_context(tc.tile_pool(name="wb", bufs=12))
        wp = ctx.enter_context(tc.tile_pool(name="wp", bufs=6))
        xfp = ctx.enter_context(tc.tile_pool(name="xfp", bufs=6))
        aa = ctx.enter_context(tc.tile_pool(name="aa", bufs=18))
        sep = ctx.enter_context(tc.tile_pool(name="sep", bufs=1))
        sm = ctx.enter_context(tc.tile_pool(name="sm", bufs=1))
        ps = ctx.enter_context(tc.tile_pool(name="ps", bufs=1, space="PSUM"))
        dram = ctx.enter_context(tc.tile_pool(name="dram", bufs=1, space="DRAM"))

        # psum helper: rotate through mm(2) + acc0..5(1) for general groups
        ps_cycle = ['mm', 'acc0', 'mm', 'acc1', 'mm', 'acc2',
                    'mm', 'acc3', 'mm', 'acc4', 'mm', 'acc5']
        ps_i = [0]

        def pt(tag=None):
            if tag is None:
                tag = ps_cycle[ps_i[0] % len(ps_cycle)]
                ps_i[0] += 1
            return ps.tile([128, 512], FP32, tag=tag,
                           bufs=(2 if tag == 'mm' else 1), name="pst")

        # ---------------- x load (first on the sync queue) ----------------
        xf = [xfp.tile([128, T], FP32, tag="xf", name=f"xf{ci}")
              for ci in range(NCH)]
        for ci in range(NCH):
            nc.sync.dma_start(xf[ci][:], xT[ci * 128:(ci + 1) * 128, :])

        # ---------------- packed constants (2 DMAs) ----------------
        auxf_t = cst.tile([128, 48], FP32, tag="auxf", name="auxf")
        nc.sync.dma_start(auxf_t[:], auxf[:])
        auxb_t = cst.tile([128, 1253], BF16, tag="auxb", name="auxb")
        nc.sync.dma_start(auxb_t[:], auxb[:])
        ones_t = auxb_t[:, 0:1]
        id_t = auxb_t[:, 1:129]
        e00_t = auxb_t[:, 129:237]
        bd9_t = auxb_t[:, 237:249]
        onesrow_t = auxb_t[:, 249:377]
        bd9T_t = auxb_t[:, 377:485]
        vbrow_t = auxb_t[:, 485:1253]
        qb_t = [auxf_t[:, j:j + 1] for j in range(12)]
        pb_t = [auxf_t[:, 12 + j:13 + j] for j in range(NCH)]
        f1b_t = [auxf_t[:, 18 + j:19 + j] for j in range(NFF)]
        f2b_t = [auxf_t[:, 42 + j:43 + j] for j in range(NCH)]

        # ---------------- weight prefetch ----------------
        # qkv q/k: 6 x [128,2304] in the big ring, column-group-ordered DMAs
        wqkv = [wb.tile([128, 3 * C], BF16, tag="w", name=f"wqkv{ci}",
                        padded_shape=[128, DFF]) for ci in range(NCH)]
        for cg in range(0, 3 * C, 512):
            gw = min(512, 3 * C - cg)
            for ci in range(NCH):
                nc.sync.dma_start(wqkv[ci][:, cg:cg + gw],
                                  qkvWt[ci * 128:(ci + 1) * 128, cg:cg + gw])
        wpj = [wp.tile([128, C], BF16, tag="wp", name=f"wpj{ci}")
               for ci in range(NCH)]
        for ci in range(NCH):
            nc.sync.dma_start(wpj[ci][:], projWt[ci * 128:(ci + 1) * 128, :])
        wf1 = [wb.tile([128, DFF], BF16, tag="w", name=f"wf1{ci}")
               for ci in range(NCH)]
        for ci in range(NCH):
            nc.sync.dma_start(wf1[ci][:], fc1Wt[ci * 128:(ci + 1) * 128, :])

        # ---------------- casts for LN1 stats ----------------
        xb = [aa.tile([128, T], BF16, tag="a", name=f"xb{ci}")
              for ci in range(NCH)]
        sq = [aa.tile([128, T], BF16, tag="a", name=f"sq{ci}")
              for ci in range(NCH)]
        for ci in range(NCH):
            nc.vector.tensor_copy(xb[ci][:], xf[ci][:])
            nc.scalar.activation(sq[ci][:], xf[ci][:], AF.Square)

        # ---------------- LN helpers ----------------
        def ln_stats(srcb, srcsq, t0, t1):
            w = t1 - t0
            p = pt()
            for ci in range(NCH):
                nc.tensor.matmul(p[0:1, :w], ones_t[:, 0:1],
                                 srcb[ci][:, t0:t1],
                                 start=(ci == 0), stop=(ci == NCH - 1))
            p2 = pt()
            for ci in range(NCH):
                nc.tensor.matmul(p2[0:1, :w], ones_t[:, 0:1],
                                 srcsq[ci][:, t0:t1],
                                 start=(ci == 0), stop=(ci == NCH - 1))
            return p, p2

        def ln_chain(p, p2, t0, t1):
            w = t1 - t0
            mu = sm.tile([1, 512], FP32, tag="mu", name="mu", bufs=1)
            tmp = sm.tile([1, 512], FP32, tag="tmp", name="tmp", bufs=1)
            al = sm.tile([1, 512], BF16, tag="al", name="al", bufs=1)
            be = sm.tile([1, 512], BF16, tag="be", name="be", bufs=1)
            nc.vector.tensor_scalar_mul(mu[0:1, :w], p[0:1, :w], 1.0 / C)
            nc.vector.tensor_mul(tmp[0:1, :w], mu[0:1, :w], mu[0:1, :w])
            # tmp = mu^2 - eps  so that  var+eps = ps2/C - tmp
            nc.vector.tensor_scalar_add(tmp[0:1, :w], tmp[0:1, :w], -1e-5)
            nc.vector.scalar_tensor_tensor(p2[0:1, :w], p2[0:1, :w], 1.0 / C,
                                           tmp[0:1, :w],
                                           op0=OP.mult, op1=OP.subtract)
            nc.scalar.activation(tmp[0:1, :w], p2[0:1, :w], AF.Sqrt)
            with nc.allow_low_precision(reason="bf16 LN scale intended"):
                nc.vector.reciprocal(al[0:1, :w], tmp[0:1, :w])
                nc.vector.scalar_tensor_tensor(be[0:1, :w], mu[0:1, :w],
                                               -1.0, al[0:1, :w],
                                               op0=OP.mult, op1=OP.mult)
            return al, be

        def ln_bcast(al, be, bcA, bcB, t0, t1):
            w = t1 - t0
            for src, dst in ((al, bcA), (be, bcB)):
                psb = pt(tag='mm')
                nc.tensor.matmul(psb[:, :w], onesrow_t[0:1, :],
                                 src[0:1, :w], start=True, stop=True)
                nc.scalar.copy(dst[:, t0:t1], psb[:, :w])

        def ln_apply(srcf, dst, bcA, bcB, t0, t1):
            for ci in range(NCH):
                nc.vector.tensor_mul(dst[ci][:, t0:t1], srcf[ci][:, t0:t1],
                                     bcA[:, t0:t1])
                nc.vector.tensor_tensor(dst[ci][:, t0:t1], dst[ci][:, t0:t1],
                                        bcB[:, t0:t1], op=OP.add)

        # ---------------- LN1 ----------------
        h = [aa.tile([128, T], BF16, tag="a", name=f"h{ci}")
             for ci in range(NCH)]
        bcA1 = sm.tile([128, T], BF16, tag="bc", name="bcA1", bufs=2)
        bcB1 = sm.tile([128, T], BF16, tag="bc", name="bcB1", bufs=2)

        s_t0, s2_t0 = ln_stats(xb, sq, *TT[0])
        al0, be0 = ln_chain(s_t0, s2_t0, *TT[0])
        ln_bcast(al0, be0, bcA1, bcB1, *TT[0])
        ln_apply(xf, h, bcA1, bcB1, *TT[0])
        s_t1, s2_t1 = ln_stats(xb, sq, *TT[1])
        al1, be1 = ln_chain(s_t1, s2_t1, *TT[1])

        # ---------------- qkv (q,k feature-major) ----------------
        k_t = [aa.tile([128, T], BF16, tag="a", name=f"k{ci}")
               for ci in range(NCH)]
        q_t = [aa.tile([128, T], BF16, tag="a", name=f"q{ci}")
               for ci in range(NCH)]

        def qk_group(j, t0, t1):
            # j in 0..11: 0-5 = q couts, 6-11 = k couts
            w = t1 - t0
            dst = q_t[j] if j < NCH else k_t[j - NCH]
            p = pt()
            for ci in range(NCH):
                nc.tensor.matmul(p[:, :w], wqkv[ci][:, j * 128:(j + 1) * 128],
                                 h[ci][:, t0:t1],
                                 start=(ci == 0), stop=(ci == NCH - 1))
            if j % 2 == 0:
                nc.scalar.activation(dst[:, t0:t1], p[:, :w], AF.Identity,
                                     bias=qb_t[j][:, 0:1])
            else:
                nc.vector.tensor_scalar_add(dst[:, t0:t1], p[:, :w],
                                            qb_t[j][:, 0:1])

        for j in range(3):
            qk_group(j, *TT[0])
        # t1 broadcast + apply overlap the first qkv groups
        ln_bcast(al1, be1, bcA1, bcB1, *TT[1])
        ln_apply(xf, h, bcA1, bcB1, *TT[1])
        for j in range(3, 12):
            qk_group(j, *TT[0])
        for j in range(12):
            qk_group(j, *TT[1])

        # v bias broadcast [128, C]
        vb_bc = sm.tile([128, C], BF16, tag="vbbc", name="vb_bc")
        for cg in range(0, C, 512):
            gw = min(512, C - cg)
            psb = pt()
            nc.tensor.matmul(psb[:, :gw], onesrow_t[0:1, :],
                             vbrow_t[0:1, cg:cg + gw], start=True, stop=True)
            nc.scalar.copy(vb_bc[:, cg:cg + gw], psb[:, :gw])

        # v token-major [T, C]
        v_t = [sep.tile([128, C], BF16, tag="v", name=f"v{tb}", bufs=NTB)
               for tb in range(NTB)]
        for tb in range(NTB):
            p0, p1_ = tb * 128, min((tb + 1) * 128, T)
            pp = p1_ - p0
            for cg in range(0, C, 512):
                gw = min(512, C - cg)
                p = pt()
                for ci in range(NCH):
                    nc.tensor.matmul(p[:pp, :gw], h[ci][:, p0:p1_],
                                     wqkv[ci][:, 2 * C + cg:2 * C + cg + gw],
                                     start=(ci == 0), stop=(ci == NCH - 1))
                nc.vector.tensor_tensor(v_t[tb][:pp, cg:cg + gw], p[:pp, :gw],
                                        vb_bc[:pp, cg:cg + gw], op=OP.add)

        # =========================================================
        # sparse attention
        # =========================================================
        attnout = [sep.tile([128, T], BF16, tag="ao", name=f"ao{ci}", bufs=6)
                   for ci in range(NCH)]
        for ci in range(NCH):
            # zero pad col 789 (and 788, rewritten by the temporal patch)
            nc.vector.memzero(attnout[ci][:, T - 2:T])

        # kbd (h,j) cols / qbd (j,h) cols, block-diag by head
        kbd = [sm.tile([128, HF], BF16, tag=f"kbd{ci}", name=f"kbd{ci}")
               for ci in range(NCH)]
        qbd = [sm.tile([128, HF], BF16, tag=f"qbd{ci}", name=f"qbd{ci}")
               for ci in range(NCH)]
        for ci in range(NCH):
            nc.vector.memzero(kbd[ci][:])
            nc.vector.memzero(qbd[ci][:])
        for hh in range(H):
            ci, po = hh // 2, (hh % 2) * 64
            nc.vector.tensor_copy(kbd[ci][po:po + 64, hh * F:(hh + 1) * F],
                                  k_t[ci][po:po + 64, SPH:SPH + F])
            nc.vector.tensor_copy(qbd[ci][po:po + 64, hh:hh + 97:H],
                                  q_t[ci][po:po + 64, SPH:SPH + F])

        # vtmp_bd [108, C]: rows (h,j) = temporal v of head h at cols h*64..
        vtmp_bd = sm.tile([HF, C], BF16, tag="vtmpbd", name="vtmpbd")
        nc.vector.memzero(vtmp_bd[0:HF, :])
        for hh in range(H):
            nc.scalar.dma_start(vtmp_bd[hh * F:(hh + 1) * F,
                                        hh * 64:(hh + 1) * 64],
                                v_t[6][12:12 + F, hh * 64:(hh + 1) * 64])

        # S1/P1: all local queries vs 9 temporal keys -> p1 [108, T]
        p1 = sm.tile([HF, T], BF16, tag="p1", name="p1")
        for (t0, t1) in TT:
            w = t1 - t0
            p = pt()
            for ci in range(NCH):
                nc.tensor.matmul(p[0:HF, :w], kbd[ci][:], q_t[ci][:, t0:t1],
                                 start=(ci == 0), stop=(ci == NCH - 1))
            nc.scalar.activation(p1[0:HF, t0:t1], p[0:HF, :w], AF.Exp,
                                 scale=SCALE)

        # lsp[h,t] = sum_j p1[(h,j),t]; rlsp = 1/lsp (bf16)
        rlsp = sm.tile([H, T], BF16, tag="rlsp", name="rlsp")
        for (t0, t1) in TT:
            w = t1 - t0
            p = pt()
            nc.tensor.matmul(p[0:H, :w], bd9_t[0:HF, :], p1[0:HF, t0:t1],
                             start=True, stop=True)
            with nc.allow_low_precision(reason="bf16 softmax recip intended"):
                nc.vector.reciprocal(rlsp[0:H, t0:t1], p[0:H, :w])

        # rlsp9 [108, T] = rlsp repeated per j; p1 *= rlsp9 (pre-normalize)
        rlsp9 = sm.tile([HF, T], BF16, tag="rlsp9", name="rlsp9")
        for (t0, t1) in TT:
            w = t1 - t0
            p = pt()
            nc.tensor.matmul(p[0:HF, :w], bd9T_t[0:H, :], rlsp[0:H, t0:t1],
                             start=True, stop=True)
            nc.vector.tensor_copy(rlsp9[0:HF, t0:t1], p[0:HF, :w])
        nc.vector.tensor_mul(p1[0:HF, :], p1[0:HF, :], rlsp9[0:HF, :])

        # O1: spatial attention out
        for ci in range(NCH):
            for (t0, t1) in TSP:
                w = t1 - t0
                p = pt()
                nc.tensor.matmul(p[:, :w],
                                 vtmp_bd[0:HF, ci * 128:(ci + 1) * 128],
                                 p1[0:HF, t0:t1], start=True, stop=True)
                nc.vector.tensor_copy(attnout[ci][:, t0:t1], p[:, :w])

        # S2T/P2T: temporal queries vs all local keys, token-major [T, 108]
        p2 = [sm.tile([128, HF], BF16, tag="p2", name=f"p2{tb}", bufs=NTB)
              for tb in range(NTB)]
        for tb in range(NTB):
            p0, p1_ = tb * 128, min((tb + 1) * 128, T)
            pp = p1_ - p0
            p = pt()
            for ci in range(NCH):
                nc.tensor.matmul(p[:pp, 0:HF], k_t[ci][:, p0:p1_], qbd[ci][:],
                                 start=(ci == 0), stop=(ci == NCH - 1))
            nc.scalar.activation(p2[tb][:pp, :], p[:pp, 0:HF], AF.Exp,
                                 scale=SCALE)
        # mask token-block 6: rows 0-11 (spatial) pass, row 12 (CLS key)
        # kept only for q_j=0 on even cores, rows 13-21 (temporal+pad) zeroed
        nc.vector.tensor_mul(p2[6][0:22, :], p2[6][0:22, :], e00_t[0:22, :])

        # l2 partial [1,108]
        l2row = sm.tile([1, HF], FP32, tag="l2", name="l2row")
        p_l2 = pt()
        for tb in range(NTB):
            p0, p1_ = tb * 128, min((tb + 1) * 128, T)
            pp = p1_ - p0
            nc.tensor.matmul(p_l2[0:1, 0:HF], ones_t[:pp, 0:1],
                             p2[tb][:pp, :],
                             start=(tb == 0), stop=(tb == NTB - 1))
        nc.scalar.copy(l2row[:], p_l2[0:1, 0:HF])

        # O2 partial [9, C]
        o2 = sm.tile([F, C], FP32, tag="o2", name="o2")
        for hh in range(H):
            p = pt()
            for tb in range(NTB):
                p0, p1_ = tb * 128, min((tb + 1) * 128, T)
                pp = p1_ - p0
                nc.tensor.matmul(p[0:F, 0:64],
                                 p2[tb][:pp, hh:hh + 97:H],
                                 v_t[tb][:pp, hh * 64:(hh + 1) * 64],
                                 start=(tb == 0), stop=(tb == NTB - 1))
            nc.scalar.copy(o2[0:F, hh * 64:(hh + 1) * 64], p[0:F, 0:64])

        # pairwise AllReduce of (o2 | l2) in one [10, C] buffer
        cc_in = dram.tile([F + 1, C], FP32, tag="cc_in", name="cc_in")
        cc_out = dram.tile([F + 1, C], FP32, tag="cc_out", name="cc_out")
        groups = [[0, 1], [2, 3], [4, 5], [6, 7]]
        nc.scalar.dma_start(cc_in[0:F, :], o2[0:F, :])
        nc.scalar.dma_start(cc_in[F:F + 1, 0:HF], l2row[:])
        nc.gpsimd.collective_compute("AllReduce", mybir.AluOpType.add,
                                     replica_groups=groups,
                                     ins=[cc_in.opt()], outs=[cc_out.opt()])

        # ---------------- fc2 weights into recycled qkv slots ----------------
        w2g = [wb.tile([128, DFF], BF16, tag="w", name=f"w2g{g}")
               for g in range(NCH)]
        for g in range(NCH):
            for kk in range(4):
                cchunk = 4 * g + kk
                nc.sync.dma_start(w2g[g][:, kk * C:(kk + 1) * C],
                                  fc2Wt[cchunk * 128:(cchunk + 1) * 128, :])

        # =========================================================
        # proj t0 -> LN2 t0 -> fused fc1+fc2 t0 (collective overlapped)
        # =========================================================
        projout = [xfp.tile([128, T], FP32, tag="xf", name=f"po{ci}")
                   for ci in range(NCH)]
        pb = [sep.tile([128, T], BF16, tag="pbb", name=f"pbb{ci}", bufs=6)
              for ci in range(NCH)]
        sq2 = [aa.tile([128, T], BF16, tag="a", name=f"sq2{ci}")
               for ci in range(NCH)]
        h2 = [sep.tile([128, T], BF16, tag="h2", name=f"h2{ci}", bufs=6)
              for ci in range(NCH)]
        bcA2 = sm.tile([128, T], BF16, tag="bc", name="bcA2", bufs=2)
        bcB2 = sm.tile([128, T], BF16, tag="bc", name="bcB2", bufs=2)

        def proj_tile(t0, t1, stats_tags=None):
            # proj couts; optionally interleave LN2 stats accumulation MMs
            # (stats_tags name two free PSUM banks to pin for the sweep)
            w = t1 - t0
            sA = sB = None
            if stats_tags:
                sA, sB = pt(tag=stats_tags[0]), pt(tag=stats_tags[1])
            for j in range(NCH):
                p = pt(tag='mm')
                for ci in range(NCH):
                    nc.tensor.matmul(p[:, :w],
                                     wpj[ci][:, j * 128:(j + 1) * 128],
                                     attnout[ci][:, t0:t1],
                                     start=(ci == 0), stop=(ci == NCH - 1))
                nc.scalar.activation(projout[j][:, t0:t1], p[:, :w],
                                     AF.Identity, bias=pb_t[j][:, 0:1])
                nc.vector.tensor_scalar_add(pb[j][:, t0:t1], p[:, :w],
                                            pb_t[j][:, 0:1])
                nc.vector.tensor_mul(sq2[j][:, t0:t1], pb[j][:, t0:t1],
                                     pb[j][:, t0:t1])
                if stats_tags:
                    nc.tensor.matmul(sA[0:1, :w], ones_t[:, 0:1],
                                     pb[j][:, t0:t1], start=(j == 0),
                                     stop=(j == NCH - 1),
                                     skip_group_check=True)
                    nc.tensor.matmul(sB[0:1, :w], ones_t[:, 0:1],
                                     sq2[j][:, t0:t1], start=(j == 0),
                                     stop=(j == NCH - 1),
                                     skip_group_check=True)
            return sA, sB

        def ln2_finish(sA, sB, t0, t1):
            al2, be2 = ln_chain(sA, sB, t0, t1)
            ln_bcast(al2, be2, bcA2, bcB2, t0, t1)
            ln_apply(projout, h2, bcA2, bcB2, t0, t1)

        def ln2_stats_seq(t0, t1):
            w = t1 - t0
            p = pt(tag='mm')
            for ci in range(NCH):
                nc.tensor.matmul(p[0:1, :w], ones_t[:, 0:1], pb[ci][:, t0:t1],
                                 start=(ci == 0), stop=(ci == NCH - 1))
            p2_ = pt(tag='mm')
            for ci in range(NCH):
                nc.tensor.matmul(p2_[0:1, :w], ones_t[:, 0:1],
                                 sq2[ci][:, t0:t1],
                                 start=(ci == 0), stop=(ci == NCH - 1))
            return p, p2_

        sA0, sB0 = proj_tile(*TT[0], stats_tags=('acc0', 'acc1'))
        ln2_finish(sA0, sB0, *TT[0])

        # fused fc1+fc2: fc2 accumulates into 6 pinned PSUM banks
        def fused(t0, t1, mid_hook=None):
            w = t1 - t0
            acc = [pt(tag=f'acc{cb}') for cb in range(NCH)]
            for g in range(NFF):
                if mid_hook is not None and g == 12:
                    mid_hook()
                pf = pt(tag='mm')
                for ci in range(NCH):
                    nc.tensor.matmul(pf[:, :w],
                                     wf1[ci][:, g * 128:(g + 1) * 128],
                                     h2[ci][:, t0:t1],
                                     start=(ci == 0), stop=(ci == NCH - 1))
                hidt = sm.tile([128, 512], BF16, tag="hid", name=f"hid{g}",
                               bufs=4)
                nc.scalar.activation(hidt[:, :w], pf[:, :w], AF.Gelu,
                                     bias=f1b_t[g][:, 0:1])
                wg, kk = g // 4, g % 4
                for cb in range(NCH):
                    nc.tensor.matmul(acc[cb][:, :w],
                                     w2g[wg][:, kk * C + cb * 128:
                                             kk * C + (cb + 1) * 128],
                                     hidt[:, :w],
                                     start=(g == 0), stop=(g == NFF - 1),
                                     skip_group_check=True)
            for cb in range(NCH):
                st = sm.tile([128, 512], FP32, tag="st", name=f"st{cb}",
                             bufs=2)
                nc.vector.scalar_tensor_tensor(st[:, :w], acc[cb][:, :w],
                                               f2b_t[cb][:, 0:1],
                                               projout[cb][:, t0:t1],
                                               op0=OP.add, op1=OP.add)
                nc.sync.dma_start(outT[cb * 128:(cb + 1) * 128, t0:t1],
                                  st[:, :w])

        # deferred: collective landing -> temporal cols -> t1 of everything
        def temporal_patch():
            l2jh = sm.tile([F, H], FP32, tag="l2jh", name="l2jh")
            o2n = sm.tile([F, C], BF16, tag="o2n", name="o2n")
            # SWDGE cast-DMA f32 -> bf16 straight into o2n
            nc.gpsimd.dma_start(o2n[0:F, :], cc_out[0:F, :])
            for j in range(F):
                nc.scalar.dma_start(l2jh[j:j + 1, :],
                                    cc_out[F:F + 1, j * H:(j + 1) * H])
            nc.vector.reciprocal(l2jh[0:F, :], l2jh[0:F, :])
            for hh in range(H):
                nc.vector.tensor_scalar_mul(o2n[0:F, hh * 64:(hh + 1) * 64],
                                            o2n[0:F, hh * 64:(hh + 1) * 64],
                                            l2jh[0:F, hh:hh + 1])
            for ci in range(NCH):
                p = ps.tile([128, 512], BF16, tag='mm', bufs=2, name="pstb")
                nc.tensor.transpose(p[:, 0:F],
                                    o2n[0:F, ci * 128:(ci + 1) * 128],
                                    id_t[0:F, 0:F])
                nc.scalar.copy(attnout[ci][:, SPH:SPH + F], p[:, 0:F])
            proj_tile(*TT[1])
            sA1, sB1 = ln2_stats_seq(*TT[1])
            ln2_finish(sA1, sB1, *TT[1])

        fused(*TT[0], mid_hook=temporal_patch)
        fused(*TT[1])

    nc.compile()
    return nc


# ---------------- host side ----------------
_compiled = {}


def kernel(**inputs):
    x = np.ascontiguousarray(np.asarray(inputs['x'], np.float32))
    qkv_w = np.asarray(inputs['qkv_w'], np.float32)
    proj_w = np.asarray(inputs['proj_w'], np.float32)
    proj_b = np.asarray(inputs['proj_b'], np.float32)
    fc1_w = np.asarray(inputs['fc1_w'], np.float32)
    fc1_b = np.asarray(inputs['fc1_b'], np.float32)
    fc2_w = np.asarray(inputs['fc2_w'], np.float32)
    fc2_b = np.asarray(inputs['fc2_b'], np.float32)
    g = np.asarray(inputs['ln2_g'], np.float32)
    bb = np.asarray(inputs['ln2_b'], np.float32)

    import ml_dtypes
    bf16 = ml_dtypes.bfloat16

    # fold LN affine (g, b) into the consuming GEMMs:
    #   W @ (LNraw(x)*g + b) = (W*g) @ LNraw(x) + W@b
    qkvW = qkv_w * g[None, :]                 # [3C, C]
    qkvB = qkv_w @ bb                         # [3C]
    fc1W = fc1_w * g[None, :]
    fc1Bf = fc1_b + fc1_w @ bb

    qkvWt = np.ascontiguousarray(qkvW.T).astype(bf16)     # [C, 3C]
    projWt = np.ascontiguousarray(proj_w.T).astype(bf16)  # [C, C]
    fc1Wt = np.ascontiguousarray(fc1W.T).astype(bf16)     # [C, DFF]
    fc2Wt = np.ascontiguousarray(fc2_w.T).astype(bf16)    # [DFF, C]

    # packed fp32 biases [128, 48]
    auxf_np = np.zeros((128, 48), np.float32)
    for j in range(12):
        auxf_np[:, j] = qkvB[j * 128:(j + 1) * 128]
    for j in range(6):
        auxf_np[:, 12 + j] = proj_b[j * 128:(j + 1) * 128]
    for j in range(24):
        auxf_np[:, 18 + j] = fc1Bf[j * 128:(j + 1) * 128]
    for j in range(6):
        auxf_np[:, 42 + j] = fc2_b[j * 128:(j + 1) * 128]

    # packed bf16 constants [128, 1253]:
    # ones(0) | ident(1:129) | e00(129:237) | bd9(237:249) |
    # onesrow(249:377) | bd9T(377:485) | vbrow(485:1253)
    bd9_np = np.zeros((H * F, H), np.float32)
    for hh in range(H):
        bd9_np[hh * F:(hh + 1) * F, hh] = 1.0
    auxb_np = np.zeros((128, 1253), np.float32)
    auxb_np[:, 0] = 1.0
    auxb_np[:, 1:129] = np.eye(128)
    # e00: multiplicative mask for p2 token-block 6 (local tokens 768..789):
    # rows 0-11 = spatial -> 1; row 12 = CLS key -> keep only q_j=0 cols
    # (cols 0..11 in (j,h) order) on even cores; rows 13-21 -> 0
    auxb_np[0:12, 129:237] = 1.0
    auxb_np[0:108, 237:249] = bd9_np
    auxb_np[0, 249:377] = 1.0
    auxb_np[0:12, 377:485] = bd9_np.T
    auxb_np[0, 485:1253] = qkvB[2 * C:]
    auxb_even = auxb_np.copy()
    auxb_even[12, 129 + 0:129 + H] = 1.0    # CLS self-term on even cores

    in_maps = []
    for core in range(8):
        b_, half = core // 2, core % 2
        sp = x[b_, F + half * SPH: F + (half + 1) * SPH]     # [780, C]
        tmp = x[b_, 0:F]                                     # [9, C]
        pad = np.zeros((1, C), np.float32)
        xTn = np.ascontiguousarray(
            np.concatenate([sp, tmp, pad], 0).T)             # [C, 790]
        in_maps.append(dict(
            xT=xTn, qkvWt=qkvWt, projWt=projWt, fc1Wt=fc1Wt, fc2Wt=fc2Wt,
            auxf=auxf_np,
            auxb=(auxb_even if half == 0 else auxb_np).astype(bf16)))

    if 'nc' not in _compiled:
        _compiled['nc'] = build_kernel()
    nc = _compiled['nc']
    res = run_bass_kernel_spmd(nc, in_maps, list(range(8)))
    _compiled['last_result'] = res

    out = np.zeros((B, N, C), np.float32)
    for core in range(8):
        b_, half = core // 2, core % 2
        oT = res.results[core]['outT']                       # [C, 790]
        if half == 0:
            out[b_, 0:F] = oT[:, SPH:SPH + F].T
            out[b_, F:F + SPH] = oT[:, 0:SPH].T
        else:
            out[b_, F + SPH:N] = oT[:, 0:SPH].T
    return out


if __name__ == '__main__':
    from reference import setup_inputs, reference
    inputs = {k: np.asarray(v) for k, v in setup_inputs().items()}
    out = kernel(**inputs)
    print("kernel ran, out shape", out.shape)


# revision 26
# speedup vs baseline: 2.1922x; 1.1846x over previous
"""Trainium2 Bass kernel for the sparse-attention ('interact' mask) transformer block.

Reference (B=4, N=1569, C=768, H=12, d=64, Dff=3072, F=9):
    h = LN(x)*g+b; qkv = h @ qkv_w.T; sparse attention (spatial rows attend
    only to the 9 temporal tokens, temporal rows attend to the 1560 spatial
    tokens, CLS also to itself); out = attn @ proj_w.T + proj_b;
    return out + MLP(LN(out)*g+b)

Sharding: 8 cores = 4 batches x 2 halves; local layout [780 spatial | 9
temporal | 1 zero pad] = 790 cols, feature-major [C, T] on chip.  Only
communication: pairwise AllReduce(add) of flash-style partials (l2, O2) for
the 9 temporal query rows, overlapped with the MLP GEMMs.

v2 design (vs. 479us baseline):
  - ALL matmul moving operands are bf16 (fp32r emitted fp32_mode=HIGH
    multi-pass: ~3x slower; measured).  LN g/b folded into weights/biases
    on the host so the device computes raw LN.
  - 2 token tiles (512+278) instead of 3 (512+268+10): the 10-col tile
    cost ~200ns/MM x 396 MMs = 83us of pure overhead.
  - weights resident/prefetched in an SBUF slot ring (no DMA serialization).
  - fc1+fc2 fused per hidden chunk (fc2 accumulates in 6 pinned PSUM banks)
    so no [128,T]x24 hid buffer and a dense warm PE stream.
  - second token tile of proj/LN2/MLP deferred behind the collective.
"""

import numpy as np
import sys
from contextlib import ExitStack

sys.path.insert(0, '/opt/trn_rl_repo')

import concourse.bass as bass
import concourse.bacc as bacc
import concourse.tile as tile
from concourse import mybir
from concourse.bass_utils import run_bass_kernel_spmd

# ---------------- problem constants ----------------
B, N, C = 4, 1569, 768
H, D = 12, 64
F = 9
DFF = 4 * C
NSP = N - F
SPH = NSP // 2
T = SPH + F + 1          # 790 local cols: [780 spatial | 9 temporal | 1 pad]
NCH = C // 128           # 6
NFF = DFF // 128         # 24
HF = H * F               # 108
NTB = (T + 127) // 128   # 7 token blocks (last = 22 rows)
SCALE = D ** -0.5

FP32 = mybir.dt.float32
BF16 = mybir.dt.bfloat16
AF = mybir.ActivationFunctionType
OP = mybir.AluOpType

TT = [(0, 512), (512, T)]        # main 2-tile split
TSP = [(0, 512), (512, SPH)]     # spatial-only (O1)


def build_kernel():
    nc = bacc.Bacc("TRN2", target_bir_lowering=False, debug=False,
                   num_devices=8)

    # ---------------- DRAM I/O ----------------
    xT = nc.dram_tensor("xT", [C, T], BF16, kind="ExternalInput")
    qkvWt = nc.dram_tensor("qkvWt", [C, 3 * C], BF16, kind="ExternalInput")
    projWt = nc.dram_tensor("projWt", [C, C], BF16, kind="ExternalInput")
    fc1Wt = nc.dram_tensor("fc1Wt", [C, DFF], BF16, kind="ExternalInput")
    fc2Wt = nc.dram_tensor("fc2Wt", [DFF, C], BF16, kind="ExternalInput")
    # packed constants: auxf [128,48] fp32 = biases (qk 0:12 | proj 12:18 |
    # fc1 18:42 | fc2 42:48); auxb [128,1253] bf16 = ones(0) | ident(1:129) |
    # e00(129:237) | bd9(237:249) | onesrow(249:377) | bd9T(377:485) |
    # vbrow(485:1253)
    auxf = nc.dram_tensor("auxf", [128, 48], FP32, kind="ExternalInput")
    auxb = nc.dram_tensor("auxb", [128, 1253], BF16, kind="ExternalInput")
    outT = nc.dram_tensor("outT", [C, T], FP32, kind="ExternalOutput")

    with tile.TileContext(nc) as tc, ExitStack() as ctx:
        cst = ctx.enter_context(tc.tile_pool(name="cst", bufs=1))
        wb = ctx.enter_context(tc.tile_pool(name="wb", bufs=12))
        wp = ctx.enter_context(tc.tile_pool(name="wp", bufs=6))
        xfp = ctx.enter_context(tc.tile_pool(name="xfp", bufs=6))
        aa = ctx.enter_context(tc.tile_pool(name="aa", bufs=18))
        sep = ctx.enter_context(tc.tile_pool(name="sep", bufs=1))
        sm = ctx.enter_context(tc.tile_pool(name="sm", bufs=1))
        ps = ctx.enter_context(tc.tile_pool(name="ps", bufs=1, space="PSUM"))
        dram = ctx.enter_context(tc.tile_pool(name="dram", bufs=1, space="DRAM"))

        # psum helper: rotate through mm(2) + acc0..5(1) for general groups
        ps_cycle = ['mm', 'acc0', 'mm', 'acc1', 'mm', 'acc2',
                    'mm', 'acc3', 'mm', 'acc4', 'mm', 'acc5']
        ps_i = [0]

        def pt(tag=None):
            if tag is None:
                tag = ps_cycle[ps_i[0] % len(ps_cycle)]
                ps_i[0] += 1
            return ps.tile([128, 512], FP32, tag=tag,
                           bufs=(2 if tag == 'mm' else 1), name="pst")

        # ---------------- x load (first on the sync queue, bf16) ----------
        xb = [aa.tile([128, T], BF16, tag="a", name=f"xb{ci}")
              for ci in range(NCH)]
        for ci in range(NCH):
            nc.sync.dma_start(xb[ci][:], xT[ci * 128:(ci + 1) * 128, :])

        # ---------------- packed constants (2 DMAs) ----------------
        auxf_t = cst.tile([128, 48], FP32, tag="auxf", name="auxf")
        nc.sync.dma_start(auxf_t[:], auxf[:])
        auxb_t = cst.tile([128, 1253], BF16, tag="auxb", name="auxb")
        nc.sync.dma_start(auxb_t[:], auxb[:])
        ones_t = auxb_t[:, 0:1]
        id_t = auxb_t[:, 1:129]
        e00_t = auxb_t[:, 129:237]
        bd9_t = auxb_t[:, 237:249]
        onesrow_t = auxb_t[:, 249:377]
        bd9T_t = auxb_t[:, 377:485]
        vbrow_t = auxb_t[:, 485:1253]
        qb_t = [auxf_t[:, j:j + 1] for j in range(12)]
        pb_t = [auxf_t[:, 12 + j:13 + j] for j in range(NCH)]
        f1b_t = [auxf_t[:, 18 + j:19 + j] for j in range(NFF)]
        f2b_t = [auxf_t[:, 42 + j:43 + j] for j in range(NCH)]

        # ---------------- weight prefetch ----------------
        # qkv q/k: 6 x [128,2304] in the big ring, column-group-ordered DMAs
        wqkv = [wb.tile([128, 3 * C], BF16, tag="w", name=f"wqkv{ci}",
                        padded_shape=[128, DFF]) for ci in range(NCH)]
        for cg in range(0, 3 * C, 512):
            gw = min(512, 3 * C - cg)
            for ci in range(NCH):
                nc.sync.dma_start(wqkv[ci][:, cg:cg + gw],
                                  qkvWt[ci * 128:(ci + 1) * 128, cg:cg + gw])
        wpj = [wp.tile([128, C], BF16, tag="wp", name=f"wpj{ci}")
               for ci in range(NCH)]
        for ci in range(NCH):
            nc.sync.dma_start(wpj[ci][:], projWt[ci * 128:(ci + 1) * 128, :])
        wf1 = [wb.tile([128, DFF], BF16, tag="w", name=f"wf1{ci}")
               for ci in range(NCH)]
        for ci in range(NCH):
            nc.sync.dma_start(wf1[ci][:], fc1Wt[ci * 128:(ci + 1) * 128, :])

        # ---------------- x^2 for LN1 stats ----------------
        sq = [aa.tile([128, T], BF16, tag="a", name=f"sq{ci}")
              for ci in range(NCH)]
        for ci in range(NCH):
            nc.scalar.activation(sq[ci][:], xb[ci][:], AF.Square)

        # ---------------- LN helpers ----------------
        def ln_stats(srcb, srcsq, t0, t1):
            w = t1 - t0
            p = pt()
            for ci in range(NCH):
                nc.tensor.matmul(p[0:1, :w], ones_t[:, 0:1],
                                 srcb[ci][:, t0:t1],
                                 start=(ci == 0), stop=(ci == NCH - 1))
            p2 = pt()
            for ci in range(NCH):
                nc.tensor.matmul(p2[0:1, :w], ones_t[:, 0:1],
                                 srcsq[ci][:, t0:t1],
                                 start=(ci == 0), stop=(ci == NCH - 1))
            return p, p2

        def ln_chain(p, p2, t0, t1):
            w = t1 - t0
            mu = sm.tile([1, 512], FP32, tag="mu", name="mu", bufs=1)
            tmp = sm.tile([1, 512], FP32, tag="tmp", name="tmp", bufs=1)
            al = sm.tile([1, 512], BF16, tag="al", name="al", bufs=1)
            be = sm.tile([1, 512], BF16, tag="be", name="be", bufs=1)
            nc.vector.tensor_scalar_mul(mu[0:1, :w], p[0:1, :w], 1.0 / C)
            nc.vector.tensor_mul(tmp[0:1, :w], mu[0:1, :w], mu[0:1, :w])
            # tmp = mu^2 - eps  so that  var+eps = ps2/C - tmp
            nc.vector.tensor_scalar_add(tmp[0:1, :w], tmp[0:1, :w], -1e-5)
            nc.vector.scalar_tensor_tensor(p2[0:1, :w], p2[0:1, :w], 1.0 / C,
                                           tmp[0:1, :w],
                                           op0=OP.mult, op1=OP.subtract)
            nc.scalar.activation(tmp[0:1, :w], p2[0:1, :w], AF.Sqrt)
            with nc.allow_low_precision(reason="bf16 LN scale intended"):
                nc.vector.reciprocal(al[0:1, :w], tmp[0:1, :w])
                nc.vector.scalar_tensor_tensor(be[0:1, :w], mu[0:1, :w],
                                               -1.0, al[0:1, :w],
                                               op0=OP.mult, op1=OP.mult)
            return al, be

        def ln_bcast(al, be, bcA, bcB, t0, t1):
            w = t1 - t0
            for src, dst in ((al, bcA), (be, bcB)):
                psb = pt(tag='mm')
                nc.tensor.matmul(psb[:, :w], onesrow_t[0:1, :],
                                 src[0:1, :w], start=True, stop=True)
                nc.scalar.copy(dst[:, t0:t1], psb[:, :w])

        def ln_apply(srcf, dst, bcA, bcB, t0, t1):
            for ci in range(NCH):
                nc.vector.tensor_mul(dst[ci][:, t0:t1], srcf[ci][:, t0:t1],
                                     bcA[:, t0:t1])
                nc.vector.tensor_tensor(dst[ci][:, t0:t1], dst[ci][:, t0:t1],
                                        bcB[:, t0:t1], op=OP.add)

        # ---------------- LN1 ----------------
        h = [aa.tile([128, T], BF16, tag="a", name=f"h{ci}")
             for ci in range(NCH)]
        bcA1 = sm.tile([128, T], BF16, tag="bc", name="bcA1", bufs=2)
        bcB1 = sm.tile([128, T], BF16, tag="bc", name="bcB1", bufs=2)

        s_t0, s2_t0 = ln_stats(xb, sq, *TT[0])
        al0, be0 = ln_chain(s_t0, s2_t0, *TT[0])
        ln_bcast(al0, be0, bcA1, bcB1, *TT[0])
        ln_apply(xb, h, bcA1, bcB1, *TT[0])
        s_t1, s2_t1 = ln_stats(xb, sq, *TT[1])
        al1, be1 = ln_chain(s_t1, s2_t1, *TT[1])

        # ---------------- qkv (q,k feature-major) ----------------
        k_t = [aa.tile([128, T], BF16, tag="a", name=f"k{ci}")
               for ci in range(NCH)]
        q_t = [aa.tile([128, T], BF16, tag="a", name=f"q{ci}")
               for ci in range(NCH)]

        def qk_group(j, t0, t1):
            # j in 0..11: 0-5 = q couts, 6-11 = k couts
            w = t1 - t0
            dst = q_t[j] if j < NCH else k_t[j - NCH]
            p = pt()
            for ci in range(NCH):
                nc.tensor.matmul(p[:, :w], wqkv[ci][:, j * 128:(j + 1) * 128],
                                 h[ci][:, t0:t1],
                                 start=(ci == 0), stop=(ci == NCH - 1))
            if j % 2 == 0:
                nc.scalar.activation(dst[:, t0:t1], p[:, :w], AF.Identity,
                                     bias=qb_t[j][:, 0:1])
            else:
                nc.vector.tensor_scalar_add(dst[:, t0:t1], p[:, :w],
                                            qb_t[j][:, 0:1])

        # k first, then q-t1 (what the collective-critical p2 path needs);
        # q-t0 is deferred until after the collective launches
        for j in range(6, 9):
            qk_group(j, *TT[0])
        # t1 broadcast + apply overlap the first qkv groups
        ln_bcast(al1, be1, bcA1, bcB1, *TT[1])
        ln_apply(xb, h, bcA1, bcB1, *TT[1])
        for j in range(9, 12):
            qk_group(j, *TT[0])
        for j in range(6, 12):
            qk_group(j, *TT[1])
        for j in range(6):
            qk_group(j, *TT[1])

        # v bias broadcast [128, C]
        vb_bc = sm.tile([128, C], BF16, tag="vbbc", name="vb_bc")
        for cg in range(0, C, 512):
            gw = min(512, C - cg)
            psb = pt()
            nc.tensor.matmul(psb[:, :gw], onesrow_t[0:1, :],
                             vbrow_t[0:1, cg:cg + gw], start=True, stop=True)
            nc.scalar.copy(vb_bc[:, cg:cg + gw], psb[:, :gw])

        # v token-major [T, C]
        v_t = [sep.tile([128, C], BF16, tag="v", name=f"v{tb}", bufs=NTB)
               for tb in range(NTB)]
        for tb in range(NTB):
            p0, p1_ = tb * 128, min((tb + 1) * 128, T)
            pp = p1_ - p0
            for cg in range(0, C, 512):
                gw = min(512, C - cg)
                p = pt()
                for ci in range(NCH):
                    nc.tensor.matmul(p[:pp, :gw], h[ci][:, p0:p1_],
                                     wqkv[ci][:, 2 * C + cg:2 * C + cg + gw],
                                     start=(ci == 0), stop=(ci == NCH - 1))
                nc.vector.tensor_tensor(v_t[tb][:pp, cg:cg + gw], p[:pp, :gw],
                                        vb_bc[:pp, cg:cg + gw], op=OP.add)

        # =========================================================
        # sparse attention — collective-critical path (p2/l2/O2) FIRST
        # =========================================================
        attnout = [sep.tile([128, T], BF16, tag="ao", name=f"ao{ci}", bufs=6)
                   for ci in range(NCH)]
        for ci in range(NCH):
            # zero pad col 789 (and 788, rewritten by the temporal patch)
            nc.vector.memzero(attnout[ci][:, T - 2:T])

        # kbd (h,j) cols / qbd (j,h) cols, block-diag by head
        kbd = [sm.tile([128, HF], BF16, tag=f"kbd{ci}", name=f"kbd{ci}")
               for ci in range(NCH)]
        qbd = [sm.tile([128, HF], BF16, tag=f"qbd{ci}", name=f"qbd{ci}")
               for ci in range(NCH)]
        for ci in range(NCH):
            nc.vector.memzero(kbd[ci][:])
            nc.vector.memzero(qbd[ci][:])
        for hh in range(H):
            ci, po = hh // 2, (hh % 2) * 64
            nc.vector.tensor_copy(qbd[ci][po:po + 64, hh:hh + 97:H],
                                  q_t[ci][po:po + 64, SPH:SPH + F])
            nc.vector.tensor_copy(kbd[ci][po:po + 64, hh * F:(hh + 1) * F],
                                  k_t[ci][po:po + 64, SPH:SPH + F])

        # S2T/P2T: temporal queries vs all local keys, token-major [T, 108]
        p2 = [sm.tile([128, HF], BF16, tag="p2", name=f"p2{tb}", bufs=NTB)
              for tb in range(NTB)]
        for tb in range(NTB):
            p0, p1_ = tb * 128, min((tb + 1) * 128, T)
            pp = p1_ - p0
            p = pt()
            for ci in range(NCH):
                nc.tensor.matmul(p[:pp, 0:HF], k_t[ci][:, p0:p1_], qbd[ci][:],
                                 start=(ci == 0), stop=(ci == NCH - 1))
            nc.scalar.activation(p2[tb][:pp, :], p[:pp, 0:HF], AF.Exp,
                                 scale=SCALE)
        # mask token-block 6: rows 0-11 (spatial) pass, row 12 (CLS key)
        # kept only for q_j=0 on even cores, rows 13-21 (temporal+pad) zeroed
        nc.vector.tensor_mul(p2[6][0:22, :], p2[6][0:22, :], e00_t[0:22, :])

        # l2 partial [1,108]
        l2row = sm.tile([1, HF], FP32, tag="l2", name="l2row")
        p_l2 = pt()
        for tb in range(NTB):
            p0, p1_ = tb * 128, min((tb + 1) * 128, T)
            pp = p1_ - p0
            nc.tensor.matmul(p_l2[0:1, 0:HF], ones_t[:pp, 0:1],
                             p2[tb][:pp, :],
                             start=(tb == 0), stop=(tb == NTB - 1))
        nc.scalar.copy(l2row[:], p_l2[0:1, 0:HF])

        # O2 partial [9, C]
        o2 = sm.tile([F, C], FP32, tag="o2", name="o2")
        for hh in range(H):
            p = pt()
            for tb in range(NTB):
                p0, p1_ = tb * 128, min((tb + 1) * 128, T)
                pp = p1_ - p0
                nc.tensor.matmul(p[0:F, 0:64],
                                 p2[tb][:pp, hh:hh + 97:H],
                                 v_t[tb][:pp, hh * 64:(hh + 1) * 64],
                                 start=(tb == 0), stop=(tb == NTB - 1))
            nc.scalar.copy(o2[0:F, hh * 64:(hh + 1) * 64], p[0:F, 0:64])

        # pairwise AllReduce of (o2 | l2) in one [10, C] buffer — launched
        # as early as possible; consumed in the fused-t0 mid hook
        cc_in = dram.tile([F + 1, C], FP32, tag="cc_in", name="cc_in")
        cc_out = dram.tile([F + 1, C], FP32, tag="cc_out", name="cc_out")
        groups = [[0, 1], [2, 3], [4, 5], [6, 7]]
        nc.scalar.dma_start(cc_in[0:F, :], o2[0:F, :])
        nc.scalar.dma_start(cc_in[F:F + 1, 0:HF], l2row[:])
        nc.gpsimd.collective_compute("AllReduce", mybir.AluOpType.add,
                                     replica_groups=groups,
                                     ins=[cc_in.opt()], outs=[cc_out.opt()])

        # ---------------- rest of qkv: q @ t0 ----------------
        for j in range(6):
            qk_group(j, *TT[0])

        # vtmp_bd [108, C]: rows (h,j) = temporal v of head h at cols h*64..
        vtmp_bd = sm.tile([HF, C], BF16, tag="vtmpbd", name="vtmpbd")
        nc.vector.memzero(vtmp_bd[0:HF, :])
        for hh in range(H):
            nc.sync.dma_start(vtmp_bd[hh * F:(hh + 1) * F,
                                      hh * 64:(hh + 1) * 64],
                              v_t[6][12:12 + F, hh * 64:(hh + 1) * 64])

        # S1/P1: all local queries vs 9 temporal keys -> p1 [108, T]
        p1 = sm.tile([HF, T], BF16, tag="p1", name="p1")
        for (t0, t1) in TT:
            w = t1 - t0
            p = pt()
            for ci in range(NCH):
                nc.tensor.matmul(p[0:HF, :w], kbd[ci][:], q_t[ci][:, t0:t1],
                                 start=(ci == 0), stop=(ci == NCH - 1))
            nc.scalar.activation(p1[0:HF, t0:t1], p[0:HF, :w], AF.Exp,
                                 scale=SCALE)

        # lsp[h,t] = sum_j p1[(h,j),t]; rlsp = 1/lsp (bf16)
        rlsp = sm.tile([H, T], BF16, tag="rlsp", name="rlsp")
        for (t0, t1) in TT:
            w = t1 - t0
            p = pt()
            nc.tensor.matmul(p[0:H, :w], bd9_t[0:HF, :], p1[0:HF, t0:t1],
                             start=True, stop=True)
            with nc.allow_low_precision(reason="bf16 softmax recip intended"):
                nc.vector.reciprocal(rlsp[0:H, t0:t1], p[0:H, :w])

        # rlsp9 [108, T] = rlsp repeated per j; p1 *= rlsp9 (pre-normalize)
        rlsp9 = sm.tile([HF, T], BF16, tag="rlsp9", name="rlsp9")
        for (t0, t1) in TT:
            w = t1 - t0
            p = pt()
            nc.tensor.matmul(p[0:HF, :w], bd9T_t[0:H, :], rlsp[0:H, t0:t1],
                             start=True, stop=True)
            nc.vector.tensor_copy(rlsp9[0:HF, t0:t1], p[0:HF, :w])
        nc.vector.tensor_mul(p1[0:HF, :], p1[0:HF, :], rlsp9[0:HF, :])

        # O1: spatial attention out
        for ci in range(NCH):
            for (t0, t1) in TSP:
                w = t1 - t0
                p = pt()
                nc.tensor.matmul(p[:, :w],
                                 vtmp_bd[0:HF, ci * 128:(ci + 1) * 128],
                                 p1[0:HF, t0:t1], start=True, stop=True)
                nc.vector.tensor_copy(attnout[ci][:, t0:t1], p[:, :w])

        # ---------------- fc2 weights into recycled qkv slots ----------------
        w2g = [wb.tile([128, DFF], BF16, tag="w", name=f"w2g{g}")
               for g in range(NCH)]
        for g in range(NCH):
            for kk in range(4):
                cchunk = 4 * g + kk
                nc.sync.dma_start(w2g[g][:, kk * C:(kk + 1) * C],
                                  fc2Wt[cchunk * 128:(cchunk + 1) * 128, :])

        # =========================================================
        # proj t0 -> LN2 t0 -> fused fc1+fc2 t0 (collective overlapped)
        # =========================================================
        projout = [xfp.tile([128, T], FP32, tag="xf", name=f"po{ci}")
                   for ci in range(NCH)]
        pb = [sep.tile([128, T], BF16, tag="pbb", name=f"pbb{ci}", bufs=6)
              for ci in range(NCH)]
        sq2 = [aa.tile([128, T], BF16, tag="a", name=f"sq2{ci}")
               for ci in range(NCH)]
        h2 = [sep.tile([128, T], BF16, tag="h2", name=f"h2{ci}", bufs=6)
              for ci in range(NCH)]
        bcA2 = sm.tile([128, T], BF16, tag="bc", name="bcA2", bufs=2)
        bcB2 = sm.tile([128, T], BF16, tag="bc", name="bcB2", bufs=2)

        def proj_tile(t0, t1, stats_tags=None):
            # proj couts; optionally interleave LN2 stats accumulation MMs
            # (stats_tags name two free PSUM banks to pin for the sweep)
            w = t1 - t0
            sA = sB = None
            if stats_tags:
                sA, sB = pt(tag=stats_tags[0]), pt(tag=stats_tags[1])
            for j in range(NCH):
                p = pt(tag='mm')
                for ci in range(NCH):
                    nc.tensor.matmul(p[:, :w],
                                     wpj[ci][:, j * 128:(j + 1) * 128],
                                     attnout[ci][:, t0:t1],
                                     start=(ci == 0), stop=(ci == NCH - 1))
                nc.scalar.activation(projout[j][:, t0:t1], p[:, :w],
                                     AF.Identity, bias=pb_t[j][:, 0:1])
                nc.vector.tensor_scalar_add(pb[j][:, t0:t1], p[:, :w],
                                            pb_t[j][:, 0:1])
                nc.vector.tensor_mul(sq2[j][:, t0:t1], pb[j][:, t0:t1],
                                     pb[j][:, t0:t1])
                if stats_tags:
                    nc.tensor.matmul(sA[0:1, :w], ones_t[:, 0:1],
                                     pb[j][:, t0:t1], start=(j == 0),
                                     stop=(j == NCH - 1),
                                     skip_group_check=True)
                    nc.tensor.matmul(sB[0:1, :w], ones_t[:, 0:1],
                                     sq2[j][:, t0:t1], start=(j == 0),
                                     stop=(j == NCH - 1),
                                     skip_group_check=True)
            return sA, sB

        def ln2_finish(sA, sB, t0, t1):
            al2, be2 = ln_chain(sA, sB, t0, t1)
            ln_bcast(al2, be2, bcA2, bcB2, t0, t1)
            ln_apply(projout, h2, bcA2, bcB2, t0, t1)

        def ln2_stats_seq(t0, t1):
            w = t1 - t0
            p = pt(tag='mm')
            for ci in range(NCH):
                nc.tensor.matmul(p[0:1, :w], ones_t[:, 0:1], pb[ci][:, t0:t1],
                                 start=(ci == 0), stop=(ci == NCH - 1))
            p2_ = pt(tag='mm')
            for ci in range(NCH):
                nc.tensor.matmul(p2_[0:1, :w], ones_t[:, 0:1],
                                 sq2[ci][:, t0:t1],
                                 start=(ci == 0), stop=(ci == NCH - 1))
            return p, p2_

        sA0, sB0 = proj_tile(*TT[0], stats_tags=('acc0', 'acc1'))
        ln2_finish(sA0, sB0, *TT[0])

        # fused fc1+fc2: fc2 accumulates into 6 pinned PSUM banks
        def fused(t0, t1, mid_hook=None):
            w = t1 - t0
            acc = [pt(tag=f'acc{cb}') for cb in range(NCH)]
            for g in range(NFF):
                if mid_hook is not None and g == 12:
                    mid_hook()
                pf = pt(tag='mm')
                for ci in range(NCH):
                    nc.tensor.matmul(pf[:, :w],
                                     wf1[ci][:, g * 128:(g + 1) * 128],
                                     h2[ci][:, t0:t1],
                                     start=(ci == 0), stop=(ci == NCH - 1))
                hidt = sm.tile([128, 512], BF16, tag="hid", name=f"hid{g}",
                               bufs=4)
                nc.scalar.activation(hidt[:, :w], pf[:, :w], AF.Gelu,
                                     bias=f1b_t[g][:, 0:1])
                wg, kk = g // 4, g % 4
                for cb in range(NCH):
                    nc.tensor.matmul(acc[cb][:, :w],
                                     w2g[wg][:, kk * C + cb * 128:
                                             kk * C + (cb + 1) * 128],
                                     hidt[:, :w],
                                     start=(g == 0), stop=(g == NFF - 1),
                                     skip_group_check=True)
            for cb in range(NCH):
                st = sm.tile([128, 512], FP32, tag="st", name=f"st{cb}",
                             bufs=2)
                nc.vector.scalar_tensor_tensor(st[:, :w], acc[cb][:, :w],
                                               f2b_t[cb][:, 0:1],
                                               projout[cb][:, t0:t1],
                                               op0=OP.add, op1=OP.add)
                nc.sync.dma_start(outT[cb * 128:(cb + 1) * 128, t0:t1],
                                  st[:, :w])

        # deferred: collective landing -> temporal cols -> t1 of everything
        def temporal_patch():
            l2jh = sm.tile([F, H], FP32, tag="l2jh", name="l2jh")
            o2n = sm.tile([F, C], BF16, tag="o2n", name="o2n")
            # SWDGE cast-DMA f32 -> bf16 straight into o2n
            nc.gpsimd.dma_start(o2n[0:F, :], cc_out[0:F, :])
            for j in range(F):
                nc.scalar.dma_start(l2jh[j:j + 1, :],
                                    cc_out[F:F + 1, j * H:(j + 1) * H])
            nc.vector.reciprocal(l2jh[0:F, :], l2jh[0:F, :])
            for hh in range(H):
                nc.vector.tensor_scalar_mul(o2n[0:F, hh * 64:(hh + 1) * 64],
                                            o2n[0:F, hh * 64:(hh + 1) * 64],
                                            l2jh[0:F, hh:hh + 1])
            for ci in range(NCH):
                p = ps.tile([128, 512], BF16, tag='mm', bufs=2, name="pstb")
                nc.tensor.transpose(p[:, 0:F],
                                    o2n[0:F, ci * 128:(ci + 1) * 128],
                                    id_t[0:F, 0:F])
                nc.scalar.copy(attnout[ci][:, SPH:SPH + F], p[:, 0:F])
            proj_tile(*TT[1])
            sA1, sB1 = ln2_stats_seq(*TT[1])
            ln2_finish(sA1, sB1, *TT[1])

        fused(*TT[0], mid_hook=temporal_patch)
        fused(*TT[1])

    nc.compile()
    return nc


# ---------------- host side ----------------
_compiled = {}


def kernel(**inputs):
    x = np.ascontiguousarray(np.asarray(inputs['x'], np.float32))
    qkv_w = np.asarray(inputs['qkv_w'], np.float32)
    proj_w = np.asarray(inputs['proj_w'], np.float32)
    proj_b = np.asarray(inputs['proj_b'], np.float32)
    fc1_w = np.asarray(inputs['fc1_w'], np.float32)
    fc1_b = np.asarray(inputs['fc1_b'], np.float32)
    fc2_w = np.asarray(inputs['fc2_w'], np.float32)
    fc2_b = np.asarray(inputs['fc2_b'], np.float32)
    g = np.asarray(inputs['ln2_g'], np.float32)
    bb = np.asarray(inputs['ln2_b'], np.float32)

    import ml_dtypes
    bf16 = ml_dtypes.bfloat16

    # fold LN affine (g, b) into the consuming GEMMs:
    #   W @ (LNraw(x)*g + b) = (W*g) @ LNraw(x) + W@b
    qkvW = qkv_w * g[None, :]                 # [3C, C]
    qkvB = qkv_w @ bb                         # [3C]
    fc1W = fc1_w * g[None, :]
    fc1Bf = fc1_b + fc1_w @ bb

    qkvWt = np.ascontiguousarray(qkvW.T).astype(bf16)     # [C, 3C]
    projWt = np.ascontiguousarray(proj_w.T).astype(bf16)  # [C, C]
    fc1Wt = np.ascontiguousarray(fc1W.T).astype(bf16)     # [C, DFF]
    fc2Wt = np.ascontiguousarray(fc2_w.T).astype(bf16)    # [DFF, C]

    # packed fp32 biases [128, 48]
    auxf_np = np.zeros((128, 48), np.float32)
    for j in range(12):
        auxf_np[:, j] = qkvB[j * 128:(j + 1) * 128]
    for j in range(6):
        auxf_np[:, 12 + j] = proj_b[j * 128:(j + 1) * 128]
    for j in range(24):
        auxf_np[:, 18 + j] = fc1Bf[j * 128:(j + 1) * 128]
    for j in range(6):
        auxf_np[:, 42 + j] = fc2_b[j * 128:(j + 1) * 128]

    # packed bf16 constants [128, 1253]:
    # ones(0) | ident(1:129) | e00(129:237) | bd9(237:249) |
    # onesrow(249:377) | bd9T(377:485) | vbrow(485:1253)
    bd9_np = np.zeros((H * F, H), np.float32)
    for hh in range(H):
        bd9_np[hh * F:(hh + 1) * F, hh] = 1.0
    auxb_np = np.zeros((128, 1253), np.float32)
    auxb_np[:, 0] = 1.0
    auxb_np[:, 1:129] = np.eye(128)
    # e00: multiplicative mask for p2 token-block 6 (local tokens 768..789):
    # rows 0-11 = spatial -> 1; row 12 = CLS key -> keep only q_j=0 cols
    # (cols 0..11 in (j,h) order) on even cores; rows 13-21 -> 0
    auxb_np[0:12, 129:237] = 1.0
    auxb_np[0:108, 237:249] = bd9_np
    auxb_np[0, 249:377] = 1.0
    auxb_np[0:12, 377:485] = bd9_np.T
    auxb_np[0, 485:1253] = qkvB[2 * C:]
    auxb_even = auxb_np.copy()
    auxb_even[12, 129 + 0:129 + H] = 1.0    # CLS self-term on even cores

    in_maps = []
    for core in range(8):
        b_, half = core // 2, core % 2
        sp = x[b_, F + half * SPH: F + (half + 1) * SPH]     # [780, C]
        tmp = x[b_, 0:F]                                     # [9, C]
        pad = np.zeros((1, C), np.float32)
        xTn = np.ascontiguousarray(
            np.concatenate([sp, tmp, pad], 0).T).astype(bf16)  # [C, 790]
        in_maps.append(dict(
            xT=xTn, qkvWt=qkvWt, projWt=projWt, fc1Wt=fc1Wt, fc2Wt=fc2Wt,
            auxf=auxf_np,
            auxb=(auxb_even if half == 0 else auxb_np).astype(bf16)))

    if 'nc' not in _compiled:
        _compiled['nc'] = build_kernel()
    nc = _compiled['nc']
    res = run_bass_kernel_spmd(nc, in_maps, list(range(8)))
    _compiled['last_result'] = res

    out = np.zeros((B, N, C), np.float32)
    for core in range(8):
        b_, half = core // 2, core % 2
        oT = res.results[core]['outT']                       # [C, 790]
        if half == 0:
            out[b_, 0:F] = oT[:, SPH:SPH + F].T
            out[b_, F:F + SPH] = oT[:, 0:SPH].T
        else:
            out[b_, F + SPH:N] = oT[:, 0:SPH].T
    return out


if __name__ == '__main__':
    from reference import setup_inputs, reference
    inputs = {k: np.asarray(v) for k, v in setup_inputs().items()}
    out = kernel(**inputs)
    print("kernel ran, out shape", out.shape)
